# revision 1
# baseline (speedup 1.0000x reference)
"""NetVLAD-style vq_codebook kernel for 8 Trainium2 NeuronCores.

Reference computation (per full input):
  assn = BN(x @ clusters); softmax over 80 clusters, drop 16 ghosts
  vlad[b,d,k] = sum_n assn[b,n,k] x[b,n,d] - a_sum[b,k]*clusters2[d,k]
  intra-normalize over d, flatten, global L2 normalize -> (B, D*K)

Sharding: data-parallel over batch B (B/8 batches per core). BatchNorm
statistics (sum and sum-of-squares per cluster column) are all-reduced
across the 8 cores (2*80 floats). Everything else is local.

Implementation notes:
 - x is cast to fp16 on load (gpsimd cast-DMA), kept in natural layout
   (token-partition) for the vlad matmul, and transposed on-chip with the
   DMA XBAR transpose into d-partition layout for the assignment matmul.
 - PE matmuls: per token tile 4 accumulating (128x128fp16)@(128x80fp16)
   matmuls for cluster assignment; vlad: per token tile one
   (128x64)@(128x512) matmul accumulating vlad^T = (64k, 512d) per batch,
   plus an N=1 matmul against a ones column for a_sum.
 - BN stats via PE: ones-column stationary matmuls against assn and
   assn^2 accumulate per-column sums in PSUM.
 - softmax without max-subtraction (logits are exactly BN-normalized,
   |logit| <~ 6, exp is safe in fp32).
"""

import sys

for _p in ("/opt/trn_rl_repo", "/root/.axon_site/_ro/trn_rl_repo"):
    if _p not in sys.path:
        sys.path.insert(0, _p)

import numpy as np

import concourse.bacc as bacc
import concourse.mybir as mybir
import concourse.tile as tile
from concourse.bass_utils import run_bass_kernel_spmd

F32 = mybir.dt.float32
F16 = mybir.dt.float16
AX = mybir.AxisListType
OP = mybir.AluOpType
ACTF = mybir.ActivationFunctionType

N_CORES = 8
D = 512
KG = 80          # clusters + ghosts
K = 64           # real clusters
N_SEQ = 2048
TPB = N_SEQ // 128   # token tiles per batch = 16
BN_EPS = 1e-5
L2_EPS = 1e-12


def build(b_loc=4, n_cores=N_CORES, with_collective=True):
    """Build the per-core program. b_loc = batches per core."""
    nt = b_loc * TPB                # token tiles per core
    tok = nt * 128                  # tokens per core
    total_tok = tok * n_cores       # global token count for BN stats

    nc = bacc.Bacc("TRN2", target_bir_lowering=False, debug=False,
                   dynamic_dma_scratch_size=65536)

    x = nc.declare_dram_parameter("x", [tok, D], F32, isOutput=False)
    cl = nc.declare_dram_parameter("clusters", [D, KG], F32, isOutput=False)
    c2 = nc.declare_dram_parameter("clusters2", [D, K], F32, isOutput=False)
    gam = nc.declare_dram_parameter("bn_gamma", [1, KG], F32, isOutput=False)
    bet = nc.declare_dram_parameter("bn_beta", [1, KG], F32, isOutput=False)
    y = nc.declare_dram_parameter("y", [b_loc, D * K], F32, isOutput=True)

    ones_row_c = nc.inline_tensor(np.ones((1, 128), np.float32), name="c_ones_row")

    with tile.TileContext(nc) as tc:
        with (
            tc.tile_pool(name="persist", bufs=1) as persist,
            tc.tile_pool(name="work", bufs=4) as work,
            tc.tile_pool(name="dram", bufs=1, space="DRAM") as dram,
        ):
            # ---- persistent SBUF tensors ----
            xh = persist.tile([128, nt, D], F16, name="xh")
            ones16 = persist.tile([128, 1], F16, name="ones16")
            assn = persist.tile([128, nt, KG], F16, name="assn")
            asqP = persist.tile([128, nt, KG], F16, name="asqP")
            sm = persist.tile([128, nt, K], F16, name="sm")
            clh = persist.tile([128, 4, KG], F16, name="clh")
            c2n = persist.tile([128, 4, K], F32, name="c2n")
            ones_row = persist.tile([1, 128], F32, name="ones_row")
            gamma = persist.tile([1, KG], F32, name="gamma")
            beta = persist.tile([1, KG], F32, name="beta")
            ss = persist.tile([1, 2 * KG], F32, name="ss")
            stats_sb = persist.tile([1, 2 * KG], F32, name="stats_sb")
            stats_g = persist.tile([1, 2 * KG], F32, name="stats_g")
            bcB = persist.tile([128, 2 * KG], F16, name="bcB")

            stats_in = dram.tile([1, 2 * KG], F32, name="stats_in")
            stats_out = dram.tile([1, 2 * KG], F32, name="stats_out")

            # ---- phase 0: constants + x load/cast ----
            nc.sync.dma_start(ones_row[:], ones_row_c.ap()[:, :])
            nc.sync.dma_start(gamma[:], gam[:, :])
            nc.sync.dma_start(beta[:], bet[:, :])
            # clusters -> fp16 chunks (cast dma): chunk c partition p = row 128c+p
            nc.gpsimd.dma_start(
                clh[:], cl.ap().rearrange("(c p) k -> p c k", p=128))
            # clusters2 natural layout; PE-transposed to (64k, 512d) below
            nc.sync.dma_start(
                c2n[:], c2.ap().rearrange("(c p) k -> p c k", p=128))
            nc.vector.memset(ones16[:], 1.0)

            # x cast-DMA in groups of 8 token tiles (SWDGE casts
            # fp32->fp16 in the DMA engines; HBM read is the real cost)
            xr = x.ap().rearrange("(t p) d -> p t d", p=128)
            for g in range(nt // 8):
                nc.gpsimd.dma_start(
                    xh[:, 8 * g:8 * (g + 1), :], xr[:, 8 * g:8 * (g + 1), :])

            # ---- phases 0b-2: transposes, assignment matmul, BN stats ----
            with tc.tile_pool(name="ps1", bufs=5, space="PSUM") as ps1:
                # BN stats accumulate in their own banks, pipelined one
                # tile-group behind the assignment matmuls (safe: start=True
                # clears has_written per-bank only)
                pstat_s = ps1.tile([1, 4 * KG], F32, name="pstat_s",
                                   tag="st_s", bufs=1)
                pstat_q = ps1.tile([1, 4 * KG], F32, name="pstat_q",
                                   tag="st_q", bufs=1)
                ng = nt // 4

                def emit_stats(g):
                    nc.tensor.matmul(pstat_s[:], ones16[:],
                                     assn[:, 4 * g:4 * g + 4, :],
                                     start=(g == 0), stop=(g == ng - 1),
                                     skip_group_check=True)
                    nc.tensor.matmul(pstat_q[:], ones16[:],
                                     asqP[:, 4 * g:4 * g + 4, :],
                                     start=(g == 0), stop=(g == ng - 1),
                                     skip_group_check=True)

                for tg in range(nt // 8):
                    xhTg = work.tile([128, 32, 128], F16, name="xhTg",
                                     tag="xhT", bufs=4)
                    # batched XBAR transpose: (128, 8*512) -> (128, 32, 128)
                    # with logical row 128*e + p at [:, e, :]; e = 4*j + c,
                    # d = 128*c + p (chunk-major per tile), matching clh
                    nc.sync.dma_start(xhTg[:, :, :],
                                      xh[:, 8 * tg:8 * (tg + 1), :],
                                      transpose=True)
                    for j in range(8):
                        t = 8 * tg + j
                        p1 = ps1.tile([128, KG], F32, name="p1", tag="p1")
                        for c in range(4):
                            nc.tensor.matmul(
                                p1[:], xhTg[:, 4 * j + c, :], clh[:, c, :],
                                start=(c == 0), stop=(c == 3),
                                skip_group_check=True)
                        nc.vector.tensor_copy(assn[:, t, :], p1[:])
                        if t % 4 == 3:
                            nc.scalar.square(asqP[:, t - 3:t + 1, :],
                                             assn[:, t - 3:t + 1, :])
                    if tg >= 1:
                        emit_stats(2 * (tg - 1))
                        emit_stats(2 * (tg - 1) + 1)
                emit_stats(ng - 2)
                emit_stats(ng - 1)


                # ---- phase 2: all-reduce stats ----
                nc.vector.tensor_reduce(
                    stats_sb[:, :KG],
                    pstat_s[:].rearrange("p (t k) -> p k t", t=4),
                    axis=AX.X, op=OP.add)
                nc.vector.tensor_reduce(
                    stats_sb[:, KG:],
                    pstat_q[:].rearrange("p (t k) -> p k t", t=4),
                    axis=AX.X, op=OP.add)

            nc.sync.dma_start(stats_in[:], stats_sb[:])
            if with_collective:
                nc.gpsimd.collective_compute(
                    "AllReduce", OP.add,
                    replica_groups=[list(range(n_cores))],
                    ins=[stats_in.opt()], outs=[stats_out.opt()])
            else:
                nc.sync.dma_start(stats_out[:], stats_in[:])
            nc.sync.dma_start(stats_g[:], stats_out[:])

            t_mean = work.tile([1, KG], F32, name="t_mean", tag="sv", bufs=6)
            t_var = work.tile([1, KG], F32, name="t_var", tag="sv", bufs=6)
            t_sd = work.tile([1, KG], F32, name="t_sd", tag="sv", bufs=6)
            t_rs = work.tile([1, KG], F32, name="t_rs", tag="sv", bufs=6)
            t_ms = work.tile([1, KG], F32, name="t_ms", tag="sv", bufs=6)
            inv_n = 1.0 / float(total_tok)
            nc.vector.tensor_scalar_mul(t_mean[:], stats_g[:, :KG], inv_n)
            nc.vector.tensor_scalar_mul(t_var[:], stats_g[:, KG:], inv_n)
            nc.vector.tensor_tensor(t_ms[:], t_mean[:], t_mean[:], op=OP.mult)
            nc.vector.tensor_tensor(t_var[:], t_var[:], t_ms[:], op=OP.subtract)
            nc.vector.tensor_scalar_add(t_var[:], t_var[:], BN_EPS)
            nc.scalar.sqrt(t_sd[:], t_var[:])
            nc.vector.reciprocal(t_rs[:], t_sd[:])
            nc.vector.tensor_tensor(ss[:, :KG], t_rs[:], gamma[:], op=OP.mult)
            nc.vector.tensor_tensor(t_ms[:], t_mean[:], ss[:, :KG], op=OP.mult)
            nc.vector.tensor_tensor(ss[:, KG:], beta[:], t_ms[:], op=OP.subtract)

            # ---- phases 3-5: softmax (all batches first, one Exp LUT
            # load), vlad matmul with x stationary -> natural (d,k) layout,
            # then normalization (software-pipelined across batches) ----
            with (
                tc.tile_pool(name="ps2", bufs=2, space="PSUM") as ps2,
                tc.tile_pool(name="elem", bufs=4) as elem,
                tc.tile_pool(name="vpost", bufs=3) as vpost,
            ):
                pbc = ps2.tile([128, 2 * KG], F32, name="pbc", tag="bc2")
                nc.tensor.matmul(pbc[:], ones_row[:], ss[:], start=True,
                                 stop=True, skip_group_check=True)
                nc.vector.tensor_copy(bcB[:], pbc[:])
                scale_b = bcB[:, :KG].rearrange("p (a k) -> p a k", a=1)
                shift_b = bcB[:, KG:].rearrange("p (a k) -> p a k", a=1)

                for b in range(b_loc):
                    t0 = b * TPB
                    te = elem.tile([128, TPB, KG], F16, name="te", tag="te")
                    nc.vector.tensor_tensor(
                        te[:], assn[:, t0:t0 + TPB, :],
                        scale_b.to_broadcast([128, TPB, KG]), op=OP.mult)
                    nc.vector.tensor_tensor(
                        te[:], te[:], shift_b.to_broadcast([128, TPB, KG]),
                        op=OP.add)
                    nc.scalar.activation(te[:], te[:], ACTF.Exp)
                    denom = work.tile([128, TPB], F16, name="denom", tag="dn")
                    with nc.allow_low_precision("fp16 softmax denom"):
                        nc.vector.tensor_reduce(denom[:], te[:], axis=AX.X,
                                                op=OP.add)
                    recip = work.tile([128, TPB], F16, name="recip", tag="rc")
                    with nc.allow_low_precision("fp16 softmax recip"):
                        nc.vector.reciprocal(recip[:], denom[:])
                    nc.vector.tensor_tensor(
                        sm[:, t0:t0 + TPB, :], te[:, :, :K],
                        recip[:].rearrange("p (t a) -> p t a", a=1)
                        .to_broadcast([128, TPB, K]), op=OP.mult)

                state = {}

                def mm_stage(b):
                    t0 = b * TPB
                    pv2 = ps2.tile([128, 4 * K], F32, name="pv2", tag="pv")
                    pas = ps2.tile([1, 4 * K], F32, name="pas", tag="pas")
                    pv3 = pv2[:].rearrange("p (c k) -> p c k", c=4)
                    # NOTE: groups must be contiguous per PSUM bank region --
                    # start=True clears has_written for the whole bank, so
                    # interleaving c-groups drops earlier partial sums.
                    for c in range(4):
                        for i in range(TPB):
                            t = t0 + i
                            nc.tensor.matmul(
                                pv3[:, c, :],
                                xh[:, t, c * 128:(c + 1) * 128],
                                sm[:, t, :],
                                start=(i == 0), stop=(i == TPB - 1),
                                skip_group_check=True)
                    for g in range(TPB // 4):
                        nc.tensor.matmul(pas[:], ones16[:],
                                         sm[:, t0 + 4 * g:t0 + 4 * g + 4, :],
                                         start=(g == 0), stop=(g == TPB // 4 - 1),
                                         skip_group_check=True)
                    state[b] = (pv2, pas)

                def post_stage(b):
                    pv2, pas = state.pop(b)
                    pv3 = pv2[:].rearrange("p (c k) -> p c k", c=4)
                    pa_sb = work.tile([1, K], F32, name="pa_sb", tag="pas_sb")
                    nc.vector.tensor_reduce(
                        pa_sb[:], pas[:].rearrange("p (i k) -> p k i", i=4),
                        axis=AX.X, op=OP.add)
                    pamB = ps2.tile([128, K], F32, name="pamB", tag="bc2")
                    nc.tensor.matmul(pamB[:], ones_row[:], pa_sb[:],
                                     start=True, stop=True,
                                     skip_group_check=True)
                    # v = vlad - a_sum*clusters2 in natural (p, c, k) layout
                    av = vpost.tile([128, 4, K], F32, name="av", tag="av")
                    nc.vector.tensor_tensor(
                        av[:], c2n[:],
                        pamB[:].rearrange("p (a k) -> p a k", a=1)
                        .to_broadcast([128, 4, K]), op=OP.mult)
                    v = vpost.tile([128, 4, K], F32, name="v", tag="v")
                    nc.vector.tensor_tensor(v[:], pv3[:], av[:],
                                            op=OP.subtract)
                    # intra-norm over d (partitions x chunks) via PE
                    sq = vpost.tile([128, 4, K], F16, name="sq", tag="sq")
                    with nc.allow_low_precision("fp16 norm squares"):
                        nc.vector.tensor_tensor(sq[:], v[:], v[:], op=OP.mult)
                    pnrm = ps2.tile([1, 4 * K], F32, name="pnrm", tag="pnrm")
                    nc.tensor.matmul(pnrm[:], ones16[:], sq[:], start=True,
                                     stop=True, skip_group_check=True)
                    nrm2 = work.tile([1, K], F32, name="nrm2", tag="nr")
                    nc.vector.tensor_reduce(
                        nrm2[:], pnrm[:].rearrange("p (c k) -> p k c", c=4),
                        axis=AX.X, op=OP.add)
                    snorm = work.tile([1, K], F32, name="snorm", tag="nr")
                    nc.scalar.sqrt(snorm[:], nrm2[:])
                    nc.vector.tensor_scalar_max(snorm[:], snorm[:], L2_EPS)
                    rn = work.tile([1, K], F32, name="rn", tag="nr")
                    nc.vector.reciprocal(rn[:], snorm[:])
                    # global norm: g2 = sum_k (snorm*rn)^2
                    t1 = work.tile([1, K], F32, name="t1", tag="nr")
                    nc.vector.tensor_tensor(t1[:], snorm[:], rn[:], op=OP.mult)
                    nc.vector.tensor_tensor(t1[:], t1[:], t1[:], op=OP.mult)
                    g2 = work.tile([1, 1], F32, name="g2", tag="g1", bufs=6)
                    nc.vector.tensor_reduce(g2[:], t1[:], axis=AX.X, op=OP.add)
                    gs = work.tile([1, 1], F32, name="gs", tag="g1", bufs=6)
                    nc.scalar.sqrt(gs[:], g2[:])
                    nc.vector.tensor_scalar_max(gs[:], gs[:], L2_EPS)
                    gr = work.tile([1, 1], F32, name="gr", tag="g1", bufs=6)
                    nc.vector.reciprocal(gr[:], gs[:])
                    nc.vector.tensor_scalar(rn[:], rn[:], gr[:], None,
                                            op0=OP.mult)
                    prnB = ps2.tile([128, K], F32, name="prnB", tag="bc2")
                    nc.tensor.matmul(prnB[:], ones_row[:], rn[:], start=True,
                                     stop=True, skip_group_check=True)
                    vf = vpost.tile([128, 4, K], F32, name="vf", tag="vf")
                    nc.vector.tensor_tensor(
                        vf[:], v[:],
                        prnB[:].rearrange("p (a k) -> p a k", a=1)
                        .to_broadcast([128, 4, K]), op=OP.mult)
                    yb = y[b, :].rearrange("(c p k) -> p c k", p=128, k=K)
                    nc.sync.dma_start(yb[:, :, :], vf[:])

                for b in range(b_loc):
                    mm_stage(b)
                    if b >= 1:
                        post_stage(b - 1)
                post_stage(b_loc - 1)
    nc.compile()
    return nc


_CACHE = {}


def _get(b_loc, n_cores, with_collective):
    key = (b_loc, n_cores, with_collective)
    if key not in _CACHE:
        _CACHE[key] = build(b_loc, n_cores, with_collective)
    return _CACHE[key]


def make_in_maps(x, clusters, clusters2, bn_gamma, bn_beta, n_cores=N_CORES):
    B = x.shape[0]
    b_loc = B // n_cores
    shared = {
        "clusters": np.ascontiguousarray(clusters, np.float32),
        "clusters2": np.ascontiguousarray(
            np.asarray(clusters2).reshape(D, K), np.float32),
        "bn_gamma": np.ascontiguousarray(
            np.asarray(bn_gamma).reshape(1, KG), np.float32),
        "bn_beta": np.ascontiguousarray(
            np.asarray(bn_beta).reshape(1, KG), np.float32),
    }
    in_maps = []
    for i in range(n_cores):
        m = dict(shared)
        m["x"] = np.ascontiguousarray(
            np.asarray(x[i * b_loc:(i + 1) * b_loc]).reshape(
                b_loc * N_SEQ, D), np.float32)
        in_maps.append(m)
    return in_maps


def kernel(x, clusters, clusters2, bn_gamma, bn_beta):
    B, N, Dd = x.shape
    assert (N, Dd) == (N_SEQ, D) and B % N_CORES == 0
    b_loc = B // N_CORES
    nc = _get(b_loc, N_CORES, True)
    in_maps = make_in_maps(x, clusters, clusters2, bn_gamma, bn_beta)
    res = run_bass_kernel_spmd(nc, in_maps, core_ids=list(range(N_CORES)))
    out = np.concatenate([res.results[i]["y"] for i in range(N_CORES)], axis=0)
    return out



# revision 32
# speedup vs baseline: 2.0093x; 2.0093x over previous
"""NetVLAD-style vq_codebook kernel for 8 Trainium2 NeuronCores.

Reference computation (per full input):
  assn = BN(x @ clusters); softmax over 80 clusters, drop 16 ghosts
  vlad[b,d,k] = sum_n assn[b,n,k] x[b,n,d] - a_sum[b,k]*clusters2[d,k]
  intra-normalize over d, flatten, global L2 normalize -> (B, D*K)

Sharding: data-parallel over batch B (B/8 batches per core). BatchNorm
statistics (sum and sum-of-squares per cluster column) are all-reduced
across the 8 cores (2*80 floats). Everything else is local.

Key structure (v2, redesigned around the engine cost model):
 - x cast-loaded fp32->fp16 by SWDGE DMA in token-partition layout.
 - x^T (d-partition) produced mostly by PE transposes (is_transpose
   matmuls writing fp16 PSUM, batch-evacuated by DVE/Act), with a few
   half-groups done by the DMA XBAR transpose to balance PE vs DMA.
 - assignment matmul per token tile: 4 accumulating (128x128)@(128x80)
   fp16 matmuls; BN stats as two long accumulating ones-matmul groups
   ([1,80] each, one emit per tile).
 - softmax: scale/shift as fp16 2x DVE tensor-tensor ops, Exp on Act
   (single activation table: ln/exp/copy only -> one table load),
   denominators reduced on gpsimd, 1/sqrt via exp(-0.5*ln(x)).
 - vlad with x stationary in a d=4p+c column layout so the final DMA
   writes 1KB-contiguous runs; a_sum accumulated directly as [1,64].
 - global L2 norm folded analytically: after intra-normalization the
   flat norm is exactly sqrt(64), so y = v * rsqrt(64*nrm2[k]).
"""

import sys

for _p in ("/opt/trn_rl_repo", "/root/.axon_site/_ro/trn_rl_repo"):
    if _p not in sys.path:
        sys.path.insert(0, _p)

import numpy as np

import concourse.bacc as bacc
import concourse.mybir as mybir
import concourse.tile as tile
from concourse.bass_utils import run_bass_kernel_spmd

F32 = mybir.dt.float32
F16 = mybir.dt.float16
AX = mybir.AxisListType
OP = mybir.AluOpType
ACTF = mybir.ActivationFunctionType

N_CORES = 8
D = 512
KG = 80          # clusters + ghosts
K = 64           # real clusters
N_SEQ = 2048
TPB = N_SEQ // 128   # token tiles per batch = 16
BN_EPS = 1e-5

# Tunables
XBAR_QUARTERS = (30, 31)  # quarter-groups (2 tiles) transposed by DMA XBAR
LAG = 4                   # software-pipeline lag (quarter-groups) for assn


def build(b_loc=4, n_cores=N_CORES, with_collective=True):
    """Build the per-core program. b_loc = batches per core."""
    nt = b_loc * TPB                # token tiles per core
    tok = nt * 128                  # tokens per core
    total_tok = tok * n_cores       # global token count for BN stats
    NH = nt // 4                    # half-groups (4 tiles each)

    nc = bacc.Bacc("TRN2", target_bir_lowering=False, debug=False,
                   dynamic_dma_scratch_size=65536)

    x = nc.declare_dram_parameter("x", [tok, D], F32, isOutput=False)
    cl = nc.declare_dram_parameter("clusters", [D, KG], F32, isOutput=False)
    c2 = nc.declare_dram_parameter("clusters2", [D, K], F32, isOutput=False)
    gam = nc.declare_dram_parameter("bn_gamma", [1, KG], F32, isOutput=False)
    bet = nc.declare_dram_parameter("bn_beta", [1, KG], F32, isOutput=False)
    y = nc.declare_dram_parameter("y", [b_loc, D * K], F32, isOutput=True)

    eye_c = nc.inline_tensor(np.eye(128, dtype=np.float16), name="c_eye")

    with tile.TileContext(nc) as tc:
        with (
            tc.tile_pool(name="persist", bufs=1) as persist,
            tc.tile_pool(name="work", bufs=4) as work,
            tc.tile_pool(name="dram", bufs=1, space="DRAM") as dram,
        ):
            # ---- persistent SBUF tensors ----
            xh = persist.tile([128, nt, D], F16, name="xh")
            assn = persist.tile([128, nt, KG], F16, name="assn")
            asq = persist.tile([128, nt, KG], F16, name="asq")
            sm = persist.tile([128, nt, K], F16, name="sm")
            idn = persist.tile([128, 128], F16, name="idn")
            clh = persist.tile([128, 4, KG], F16, name="clh")
            c2n = persist.tile([128, 4, K], F16, name="c2n")
            ones16 = persist.tile([128, 1], F16, name="ones16")
            gamma = persist.tile([1, KG], F32, name="gamma")
            beta = persist.tile([1, KG], F32, name="beta")
            ss16 = persist.tile([1, 2 * KG], F16, name="ss16")
            bcB = persist.tile([128, 2 * KG], F16, name="bcB")
            stats_sb = persist.tile([1, 2 * KG], F32, name="stats_sb")
            stats_g = persist.tile([1, 2 * KG], F32, name="stats_g")
            actwarm = persist.tile([1, 1], F32, name="actwarm")
            eps_sb = persist.tile([1, 1], F32, name="eps_sb")

            stats_in = dram.tile([1, 2 * KG], F32, name="stats_in")
            stats_out = dram.tile([1, 2 * KG], F32, name="stats_out")

            # ---- phase 0: constants + x load/cast ----
            nc.sync.dma_start(gamma[:], gam[:, :])
            nc.sync.dma_start(beta[:], bet[:, :])
            nc.sync.dma_start(idn[:], eye_c.ap()[:, :])
            nc.vector.memset(ones16[:], 1.0)
            nc.vector.memset(eps_sb[:], BN_EPS)
            # Pre-load the one activation table covering every function this
            # kernel uses (ln/exp/copy/square), so the table-load inserter
            # doesn't alternate between ln-only and exp-only sets.
            from concourse.hw_specs import get_activation_tables
            tabs = get_activation_tables(nc.m.arch)
            set_id = list(tabs).index("natural_log_exp_and_others")
            nc.scalar.add_instruction(mybir.InstLoadActFuncSet(
                name=nc.get_next_instruction_name(),
                engine=mybir.EngineType.Activation,
                act_func_set_id=set_id, ins=[], outs=[]))
            # Touch the activation engine early so any residual table load
            # happens off the critical path.
            nc.scalar.activation(actwarm[:], gamma[:, :1], ACTF.Ln)

            # x cast-DMA (SWDGE casts fp32->fp16 in the DMA engines; HBM
            # read is the real cost). Small first chunks start the PE
            # transpose pipeline sooner.
            xr = x.ap().rearrange("(t p) d -> p t d", p=128)
            t0 = 0
            for sz in (4, 4) + (8,) * ((nt - 8) // 8):
                nc.gpsimd.dma_start(
                    xh[:, t0:t0 + sz, :], xr[:, t0:t0 + sz, :])
                t0 += sz
            assert t0 == nt
            # clusters via HWDGE (fp32) + DVE cast: the Pool/SWDGE queue is
            # saturated by the x loads, and clh is needed early.
            clf = work.tile([128, 4, KG], F32, name="clf", tag="clf", bufs=1)
            nc.sync.dma_start(
                clf[:], cl.ap().rearrange("(c p) k -> p c k", p=128))
            nc.vector.tensor_copy(clh[:], clf[:])
            # clusters2 in d=4p+c layout (matches vlad output partitioning);
            # not needed until the post stage, so SWDGE order is fine.
            nc.gpsimd.dma_start(
                c2n[:], c2.ap().rearrange("(p c) k -> p c k", c=4))

            # ---- phase A: transposes + assignment matmul + BN stats ----
            with tc.tile_pool(name="psA", bufs=2, space="PSUM") as psA:
                pstat_s = psA.tile([1, KG], F32, name="pstat_s",
                                   tag="st_s", bufs=1)
                pstat_q = psA.tile([1, KG], F32, name="pstat_q",
                                   tag="st_q", bufs=1)

                NQ = nt // 2            # quarter-groups (2 tiles each)
                xtbufs = {}
                p1bufs = {}

                def produce(q):
                    # xTsb for quarter q: [128, 8, 128] fp16 with block
                    # e = 4j + c holding x[tile 2q+j, 128c:128c+128]^T
                    xTsb = work.tile([128, 8, 128], F16, name=f"xT{q}",
                                     tag="xt", bufs=LAG + 2)
                    if q in XBAR_QUARTERS:
                        nc.sync.dma_start(xTsb[:, :, :],
                                          xh[:, 2 * q:2 * (q + 1), :],
                                          transpose=True)
                    else:
                        pxt = psA.tile([128, 8, 128], F16, name="pxt",
                                       tag="pxt", bufs=4)
                        for j in range(2):
                            t = 2 * q + j
                            for c in range(4):
                                nc.tensor.transpose(
                                    pxt[:, 4 * j + c, :],
                                    xh[:, t, 128 * c:128 * (c + 1)], idn[:])
                        # batched PSUM->SBUF evacuation; alternate engines
                        if q % 3 != 2:
                            nc.vector.tensor_copy(xTsb[:], pxt[:])
                        else:
                            nc.scalar.activation(xTsb[:], pxt[:], ACTF.Copy)
                    xtbufs[q] = xTsb

                def consume(q):
                    xTsb = xtbufs.pop(q)
                    if q % 2 == 0:
                        p1bufs[q // 2] = psA.tile([128, 4, KG], F32,
                                                  name="p1", tag="p1", bufs=2)
                    p1 = p1bufs[q // 2]
                    for j in range(2):
                        for c in range(4):
                            nc.tensor.matmul(
                                p1[:, 2 * (q % 2) + j, :],
                                xTsb[:, 4 * j + c, :],
                                clh[:, c, :], start=(c == 0), stop=(c == 3),
                                skip_group_check=True)
                    if q % 2 == 1:
                        h = q // 2
                        sl = slice(4 * h, 4 * (h + 1))
                        nc.scalar.activation(assn[:, sl, :], p1[:], ACTF.Copy)
                        nc.vector.tensor_tensor(asq[:, sl, :], assn[:, sl, :],
                                                assn[:, sl, :], op=OP.mult)

                def stats(h):
                    for j in range(4):
                        t = 4 * h + j
                        nc.tensor.matmul(pstat_s[:], ones16[:],
                                         assn[:, t, :],
                                         start=(t == 0), stop=(t == nt - 1),
                                         skip_group_check=True)
                        nc.tensor.matmul(pstat_q[:], ones16[:],
                                         asq[:, t, :],
                                         start=(t == 0), stop=(t == nt - 1),
                                         skip_group_check=True)

                # Stats matmuls are emitted 3 half-groups behind the assn
                # evacuations they read: the PE queue is in-order, so a stats
                # matmul whose Act/DVE evacuation hasn't retired yet would
                # stall the whole PE pipeline.
                stats_done = 0
                for q in range(NQ + LAG):
                    if q < NQ:
                        produce(q)
                    if q >= LAG:
                        cq = q - LAG
                        consume(cq)
                        ready_h = (cq + 1) // 2 - 3
                        while stats_done < ready_h:
                            stats(stats_done)
                            stats_done += 1
                while stats_done < NH:
                    stats(stats_done)
                    stats_done += 1

                # ---- neck: stats all-reduce + BN parameters ----
                nc.vector.tensor_copy(stats_sb[:, :KG], pstat_s[:])
                nc.vector.tensor_copy(stats_sb[:, KG:], pstat_q[:])

            if with_collective:
                nc.sync.dma_start(stats_in[:], stats_sb[:])
                nc.gpsimd.collective_compute(
                    "AllReduce", OP.add,
                    replica_groups=[list(range(n_cores))],
                    ins=[stats_in.opt()], outs=[stats_out.opt()])
                nc.sync.dma_start(stats_g[:], stats_out[:])
            else:
                # single-core stand-in for the collective hop
                nc.sync.dma_start(stats_g[:], stats_sb[:])

            t_s2 = work.tile([1, KG], F32, name="t_s2", tag="sv2", bufs=4)
            t_vr = work.tile([1, KG], F32, name="t_vr", tag="sv2", bufs=4)
            t_ln = work.tile([1, KG], F32, name="t_ln", tag="sv2", bufs=4)
            t_sc = work.tile([1, KG], F32, name="t_sc", tag="sv2", bufs=4)
            t_mc = work.tile([1, KG], F32, name="t_mc", tag="sv2", bufs=4)
            inv_n = 1.0 / float(total_tok)
            # var = inv_n*(q - inv_n*s^2); rsqrt via exp(-0.5 ln(.)) with the
            # inv_n factor folded into the Ln's scale operand
            s_row, q_row = stats_g[:, :KG], stats_g[:, KG:]
            nc.vector.tensor_tensor(t_s2[:], s_row, s_row, op=OP.mult)
            nc.vector.scalar_tensor_tensor(t_vr[:], t_s2[:], -inv_n, q_row,
                                           op0=OP.mult, op1=OP.add)
            nc.scalar.activation(t_ln[:], t_vr[:], ACTF.Ln, bias=eps_sb[:],
                                 scale=inv_n)
            nc.scalar.activation(t_sc[:], t_ln[:], ACTF.Exp, scale=-0.5)
            with nc.allow_low_precision("fp16 bn scale"):
                nc.vector.tensor_tensor(ss16[:, :KG], t_sc[:], gamma[:],
                                        op=OP.mult)
            # shift = beta - (inv_n*s)*scale_f32*gamma; use fp16 scale copy
            with nc.allow_low_precision("fp16 bn shift"):
                nc.vector.scalar_tensor_tensor(t_mc[:], s_row, inv_n,
                                               ss16[:, :KG],
                                               op0=OP.mult, op1=OP.mult)
                nc.vector.tensor_tensor(ss16[:, KG:], beta[:], t_mc[:],
                                        op=OP.subtract)
            nc.gpsimd.partition_broadcast(bcB[:], ss16[:])
            scale_b = bcB[:, :KG].rearrange("p (a k) -> p a k", a=1)
            shift_b = bcB[:, KG:].rearrange("p (a k) -> p a k", a=1)

            # ---- phase BC: softmax + vlad + normalize, per batch ----
            with (
                tc.tile_pool(name="psB", bufs=2, space="PSUM") as psB,
                tc.tile_pool(name="elem", bufs=2) as elem,
                tc.tile_pool(name="vpost", bufs=2) as vpost,
            ):
                state = {}
                tebufs = {}

                def te_chunk(t0, n, pool_add=False):
                    # te = exp(scale*assn + shift) for token tiles [t0,t0+n)
                    te = elem.tile([128, n, KG], F16, name="te",
                                   tag=f"te{t0}_{n}", bufs=1)
                    nc.vector.tensor_tensor(
                        te[:], assn[:, t0:t0 + n, :],
                        scale_b.to_broadcast([128, n, KG]), op=OP.mult)
                    eng = nc.gpsimd if pool_add else nc.vector
                    eng.tensor_tensor(
                        te[:], te[:], shift_b.to_broadcast([128, n, KG]),
                        op=OP.add)
                    nc.scalar.activation(te[:], te[:], ACTF.Exp)
                    tebufs[t0] = te

                def sm_chunk(t0, n):
                    # normalize: sm = te / sum_k te, dropping ghosts
                    te = tebufs.pop(t0)
                    # pairwise-add halves at fp16 2x before the 1x reduce
                    dh = work.tile([128, n, KG // 2], F16, name="dh",
                                   tag=f"dh{n}", bufs=3)
                    with nc.allow_low_precision("fp16 softmax denom"):
                        nc.vector.tensor_tensor(dh[:], te[:, :, :KG // 2],
                                                te[:, :, KG // 2:], op=OP.add)
                    denom = work.tile([128, n], F16, name="denom", tag=f"dn{n}",
                                      bufs=3)
                    with nc.allow_low_precision("fp16 softmax denom"):
                        nc.vector.tensor_reduce(denom[:], dh[:], axis=AX.X,
                                                op=OP.add)
                    recip = work.tile([128, n], F16, name="recip", tag=f"rc{n}",
                                      bufs=3)
                    with nc.allow_low_precision("fp16 softmax recip"):
                        nc.vector.reciprocal(recip[:], denom[:])
                    nc.vector.tensor_tensor(
                        sm[:, t0:t0 + n, :], te[:, :, :K],
                        recip[:].rearrange("p (t a) -> p t a", a=1)
                        .to_broadcast([128, n, K]), op=OP.mult)

                # chunking: small first chunks so the first vlad matmuls can
                # start early; full batches later for low op overhead
                CHUNKS = [(0, 2), (2, 2), (4, 4), (8, 8)] + [
                    (b * TPB, TPB) for b in range(1, b_loc)]

                def mm_stage(b):
                    t0 = b * TPB
                    pv = psB.tile([128, 4, K], F32, name="pv", tag="pv")
                    pas = psB.tile([1, K], F32, name="pas", tag="pas")
                    # vlad: x stationary with d = 4p + c column layout
                    for c in range(4):
                        for i in range(TPB):
                            t = t0 + i
                            nc.tensor.matmul(
                                pv[:, c, :],
                                xh[:, t, c::4],
                                sm[:, t, :],
                                start=(i == 0), stop=(i == TPB - 1),
                                skip_group_check=True)
                    for i in range(TPB):
                        nc.tensor.matmul(pas[:], ones16[:], sm[:, t0 + i, :],
                                         start=(i == 0), stop=(i == TPB - 1),
                                         skip_group_check=True)
                    state[b] = (pv, pas)

                def post_stage(b):
                    pv, pas = state.pop(b)
                    pa16 = work.tile([1, K], F16, name="pa16", tag="pa16")
                    with nc.allow_low_precision("fp16 a_sum"):
                        nc.scalar.activation(pa16[:], pas[:], ACTF.Copy)
                    pamB = vpost.tile([128, K], F16, name="pamB", tag="pam")
                    nc.gpsimd.partition_broadcast(pamB[:], pa16[:])
                    av = vpost.tile([128, 4, K], F16, name="av", tag="av")
                    nc.gpsimd.tensor_tensor(
                        av[:], c2n[:],
                        pamB[:].rearrange("p (a k) -> p a k", a=1)
                        .to_broadcast([128, 4, K]), op=OP.mult)
                    v = vpost.tile([128, 4, K], F16, name="v", tag="v")
                    with nc.allow_low_precision("fp16 vlad residual"):
                        nc.vector.tensor_tensor(v[:], pv[:], av[:],
                                                op=OP.subtract)
                    sq = vpost.tile([128, 4, K], F16, name="sq", tag="sq")
                    with nc.allow_low_precision("fp16 norm squares"):
                        nc.scalar.activation(sq[:], v[:], ACTF.Square)
                    pnrm = psB.tile([1, K], F32, name="pnrm", tag="pnrm")
                    for c in range(4):
                        nc.tensor.matmul(pnrm[:], ones16[:], sq[:, c, :],
                                         start=(c == 0), stop=(c == 3),
                                         skip_group_check=True)
                    # y = v * rsqrt(64*nrm2): intra-norm and global L2 norm
                    # folded (flat norm is exactly sqrt(64) post intra-norm)
                    rnl = work.tile([1, K], F32, name="rnl", tag="rnl")
                    nc.scalar.activation(rnl[:], pnrm[:], ACTF.Ln, scale=64.0)
                    rn16 = work.tile([1, K], F16, name="rn16", tag="rn16")
                    with nc.allow_low_precision("fp16 norm scale"):
                        nc.scalar.activation(rn16[:], rnl[:], ACTF.Exp,
                                             scale=-0.5)
                    prnB = vpost.tile([128, K], F16, name="prnB", tag="prn")
                    nc.gpsimd.partition_broadcast(prnB[:], rn16[:])
                    vf = vpost.tile([128, 4, K], F32, name="vf", tag="vf")
                    nc.gpsimd.tensor_tensor(
                        vf[:], v[:],
                        prnB[:].rearrange("p (a k) -> p a k", a=1)
                        .to_broadcast([128, 4, K]), op=OP.mult)
                    yb = y[b, :].rearrange("(p c k) -> p c k", p=128, k=K)
                    nc.sync.dma_start(yb[:, :, :], vf[:])

                # Skew-by-one software pipeline: each sm chunk is emitted one
                # te-chunk later so the DVE stream never waits on an Act exp;
                # vlad (PE) and post stages weave in as batches complete.
                nch = len(CHUNKS)
                done_b = 0
                for i in range(nch + 1):
                    if i < nch:
                        t0, n = CHUNKS[i]
                        te_chunk(t0, n, pool_add=False)
                    if i >= 1:
                        t0, n = CHUNKS[i - 1]
                        sm_chunk(t0, n)
                        if (t0 + n) % TPB == 0:     # batch done_b fully sm'd
                            mm_stage(done_b)
                            if done_b >= 1:
                                post_stage(done_b - 1)
                            done_b += 1
                post_stage(b_loc - 1)
    nc.compile()
    return nc


_CACHE = {}


def _get(b_loc, n_cores, with_collective):
    key = (b_loc, n_cores, with_collective)
    if key not in _CACHE:
        _CACHE[key] = build(b_loc, n_cores, with_collective)
    return _CACHE[key]


def make_in_maps(x, clusters, clusters2, bn_gamma, bn_beta, n_cores=N_CORES):
    B = x.shape[0]
    b_loc = B // n_cores
    shared = {
        "clusters": np.ascontiguousarray(clusters, np.float32),
        "clusters2": np.ascontiguousarray(
            np.asarray(clusters2).reshape(D, K), np.float32),
        "bn_gamma": np.ascontiguousarray(
            np.asarray(bn_gamma).reshape(1, KG), np.float32),
        "bn_beta": np.ascontiguousarray(
            np.asarray(bn_beta).reshape(1, KG), np.float32),
    }
    in_maps = []
    for i in range(n_cores):
        m = dict(shared)
        m["x"] = np.ascontiguousarray(
            np.asarray(x[i * b_loc:(i + 1) * b_loc]).reshape(
                b_loc * N_SEQ, D), np.float32)
        in_maps.append(m)
    return in_maps


def kernel(x, clusters, clusters2, bn_gamma, bn_beta):
    B, N, Dd = x.shape
    assert (N, Dd) == (N_SEQ, D) and B % N_CORES == 0
    b_loc = B // N_CORES
    nc = _get(b_loc, N_CORES, True)
    in_maps = make_in_maps(x, clusters, clusters2, bn_gamma, bn_beta)
    res = run_bass_kernel_spmd(nc, in_maps, core_ids=list(range(N_CORES)))
    out = np.concatenate([res.results[i]["y"] for i in range(N_CORES)], axis=0)
    return out


# revision 48
# speedup vs baseline: 2.0282x; 1.0094x over previous
"""NetVLAD-style vq_codebook kernel for 8 Trainium2 NeuronCores.

Reference computation (per full input):
  assn = BN(x @ clusters); softmax over 80 clusters, drop 16 ghosts
  vlad[b,d,k] = sum_n assn[b,n,k] x[b,n,d] - a_sum[b,k]*clusters2[d,k]
  intra-normalize over d, flatten, global L2 normalize -> (B, D*K)

Sharding: data-parallel over batch B (B/8 batches per core). BatchNorm
statistics (sum and sum-of-squares per cluster column) are all-reduced
across the 8 cores (2*80 floats). Everything else is local.

Key structure (v2, redesigned around the engine cost model):
 - x cast-loaded fp32->fp16 by SWDGE DMA in token-partition layout.
 - x^T (d-partition) via PE transposes (is_transpose matmuls writing
   fp16 PSUM), software-pipelined at 2-tile granularity and batch-
   evacuated to SBUF by DVE and Act in a ~17:15 split.
 - assignment matmul per token tile: 4 accumulating (128x128)@(128x80)
   fp16 matmuls. BN sum-of-squares via a long PE ones-matmul group;
   BN sums via DVE free-axis reduces + a PE partition reduce, sharing
   one PSUM bank with strictly sequential accumulation groups.
 - softmax: scale/shift as fp16 2x DVE tensor-tensor ops, Exp on Act
   (one activation table for ln/exp/copy/square -> a single load),
   pairwise-halved fp16 denominator, 1/sqrt as exp(-0.5*ln(x)).
 - vlad with x stationary in a d=4p+c column layout so the final DMA
   writes 1KB-contiguous runs; a_sum accumulated directly as [1,64]
   before the vlad groups so a_sum*clusters2 overlaps them.
 - global L2 norm folded analytically: after intra-normalization the
   flat norm is exactly sqrt(64), so y = v * rsqrt(64*nrm2[k]).
 - one serial neck (stats hop + BN math) between the assignment pass
   and the softmax/vlad pass; batch-0 softmax runs in small chunks so
   the first vlad matmuls start early.
"""

import sys

for _p in ("/opt/trn_rl_repo", "/root/.axon_site/_ro/trn_rl_repo"):
    if _p not in sys.path:
        sys.path.insert(0, _p)

import numpy as np

import concourse.bacc as bacc
import concourse.mybir as mybir
import concourse.tile as tile
from concourse.bass_utils import run_bass_kernel_spmd

F32 = mybir.dt.float32
F16 = mybir.dt.float16
AX = mybir.AxisListType
OP = mybir.AluOpType
ACTF = mybir.ActivationFunctionType

N_CORES = 8
D = 512
KG = 80          # clusters + ghosts
K = 64           # real clusters
N_SEQ = 2048
TPB = N_SEQ // 128   # token tiles per batch = 16
BN_EPS = 1e-5

# Tunables
import os as _os
XBAR_QUARTERS = tuple(
    int(v) for v in _os.environ.get("K_XBAR", "").split(",") if v)
LAG = int(_os.environ.get("K_LAG", "5"))
LOADS = tuple(int(v) for v in _os.environ.get("K_LOADS", "4,4").split(","))
PXT_BUFS = int(_os.environ.get("K_PXT", "5"))


def build(b_loc=4, n_cores=N_CORES, with_collective=True):
    """Build the per-core program. b_loc = batches per core."""
    nt = b_loc * TPB                # token tiles per core
    tok = nt * 128                  # tokens per core
    total_tok = tok * n_cores       # global token count for BN stats
    NH = nt // 4                    # half-groups (4 tiles each)

    nc = bacc.Bacc("TRN2", target_bir_lowering=False, debug=False,
                   dynamic_dma_scratch_size=65536)

    x = nc.declare_dram_parameter("x", [tok, D], F32, isOutput=False)
    cl = nc.declare_dram_parameter("clusters", [D, KG], F32, isOutput=False)
    c2 = nc.declare_dram_parameter("clusters2", [D, K], F32, isOutput=False)
    gam = nc.declare_dram_parameter("bn_gamma", [1, KG], F32, isOutput=False)
    bet = nc.declare_dram_parameter("bn_beta", [1, KG], F32, isOutput=False)
    y = nc.declare_dram_parameter("y", [b_loc, D * K], F32, isOutput=True)

    eye_c = nc.inline_tensor(np.eye(128, dtype=np.float16), name="c_eye")

    with tile.TileContext(nc) as tc:
        with (
            tc.tile_pool(name="persist", bufs=1) as persist,
            tc.tile_pool(name="work", bufs=4) as work,
            tc.tile_pool(name="dram", bufs=1, space="DRAM") as dram,
        ):
            # ---- persistent SBUF tensors ----
            xh = persist.tile([128, nt, D], F16, name="xh")
            assn = persist.tile([128, nt, KG], F16, name="assn")
            asq = persist.tile([128, nt, KG], F16, name="asq")
            sm = persist.tile([128, nt, K], F16, name="sm")
            idn = persist.tile([128, 128], F16, name="idn")
            clh = persist.tile([128, 4, KG], F16, name="clh")
            c2n = persist.tile([128, 4, K], F16, name="c2n")
            ones16 = persist.tile([128, 1], F16, name="ones16")
            ones_row = persist.tile([1, 128], F16, name="ones_row")
            gamma = persist.tile([1, KG], F32, name="gamma")
            beta = persist.tile([1, KG], F32, name="beta")
            ss16 = persist.tile([1, 2 * KG], F16, name="ss16")
            bcB = persist.tile([128, 2 * KG], F16, name="bcB")
            stats_sb = persist.tile([1, 2 * KG], F32, name="stats_sb")
            stats_g = persist.tile([1, 2 * KG], F32, name="stats_g")
            actwarm = persist.tile([1, 1], F32, name="actwarm")
            eps_sb = persist.tile([1, 1], F32, name="eps_sb")

            stats_in = dram.tile([1, 2 * KG], F32, name="stats_in")
            stats_out = dram.tile([1, 2 * KG], F32, name="stats_out")

            # ---- phase 0: constants + x load/cast ----
            nc.sync.dma_start(gamma[:], gam[:, :])
            nc.sync.dma_start(beta[:], bet[:, :])
            nc.sync.dma_start(idn[:], eye_c.ap()[:, :])
            nc.vector.memset(ones16[:], 1.0)
            nc.vector.memset(ones_row[:], 1.0)
            nc.vector.memset(eps_sb[:], BN_EPS)
            # Pre-load the one activation table covering every function this
            # kernel uses (ln/exp/copy/square), so the table-load inserter
            # doesn't alternate between ln-only and exp-only sets. Best
            # effort: fall back to automatic insertion if the set is absent.
            try:
                from concourse.hw_specs import get_activation_tables
                tabs = get_activation_tables(nc.m.arch)
                set_id = list(tabs).index("natural_log_exp_and_others")
                nc.scalar.add_instruction(mybir.InstLoadActFuncSet(
                    name=nc.get_next_instruction_name(),
                    engine=mybir.EngineType.Activation,
                    act_func_set_id=set_id, ins=[], outs=[]))
            except (ImportError, ValueError, KeyError):
                pass
            # Touch the activation engine early so any residual table load
            # happens off the critical path.
            nc.scalar.activation(actwarm[:], gamma[:, :1], ACTF.Ln)

            # x cast-DMA (SWDGE casts fp32->fp16 in the DMA engines; HBM
            # read is the real cost). Small first chunks start the PE
            # transpose pipeline sooner.
            xr = x.ap().rearrange("(t p) d -> p t d", p=128)
            t0 = 0
            for sz in LOADS + (8,) * ((nt - sum(LOADS)) // 8):
                nc.gpsimd.dma_start(
                    xh[:, t0:t0 + sz, :], xr[:, t0:t0 + sz, :])
                t0 += sz
            assert t0 == nt
            # clusters via HWDGE (fp32) + DVE cast: the Pool/SWDGE queue is
            # saturated by the x loads, and clh is needed early.
            clf = work.tile([128, 4, KG], F32, name="clf", tag="clf", bufs=1)
            nc.sync.dma_start(
                clf[:], cl.ap().rearrange("(c p) k -> p c k", p=128))
            nc.vector.tensor_copy(clh[:], clf[:])
            # clusters2 in d=4p+c layout (matches vlad output partitioning);
            # not needed until the post stage, so SWDGE order is fine.
            nc.gpsimd.dma_start(
                c2n[:], c2.ap().rearrange("(p c) k -> p c k", c=4))

            # ---- phase A: transposes + assignment matmul + BN stats ----
            with tc.tile_pool(name="psA", bufs=2, space="PSUM") as psA:
                # one bank: [:,0,:] = sum-of-squares accumulation group,
                # [:,1,:] = final partition-reduce of the DVE-computed sums
                pstat = psA.tile([1, 2, KG], F32, name="pstat",
                                 tag="st", bufs=1)

                NQ = nt // 2            # quarter-groups (2 tiles each)
                xtbufs = {}
                p1bufs = {}

                def produce(q):
                    # xTsb for quarter q: [128, 8, 128] fp16 with block
                    # e = 4j + c holding x[tile 2q+j, 128c:128c+128]^T
                    xTsb = work.tile([128, 8, 128], F16, name=f"xT{q}",
                                     tag="xt", bufs=LAG + 2)
                    if q in XBAR_QUARTERS:
                        nc.sync.dma_start(xTsb[:, :, :],
                                          xh[:, 2 * q:2 * (q + 1), :],
                                          transpose=True)
                    else:
                        pxt = psA.tile([128, 8, 128], F16, name="pxt",
                                       tag="pxt", bufs=PXT_BUFS)
                        for j in range(2):
                            t = 2 * q + j
                            for c in range(4):
                                nc.tensor.transpose(
                                    pxt[:, 4 * j + c, :],
                                    xh[:, t, 128 * c:128 * (c + 1)], idn[:])
                        # batched PSUM->SBUF evacuation; split ~17:15
                        # between DVE and Act (DVE is ~1.5x faster per elem)
                        if (q * 17) // 32 != ((q - 1) * 17) // 32:
                            nc.vector.tensor_copy(xTsb[:], pxt[:])
                        else:
                            nc.scalar.activation(xTsb[:], pxt[:], ACTF.Copy)
                    xtbufs[q] = xTsb

                def consume(q):
                    xTsb = xtbufs.pop(q)
                    if q % 2 == 0:
                        p1bufs[q // 2] = psA.tile([128, 4, KG], F32,
                                                  name="p1", tag="p1", bufs=2)
                    p1 = p1bufs[q // 2]
                    for j in range(2):
                        for c in range(4):
                            nc.tensor.matmul(
                                p1[:, 2 * (q % 2) + j, :],
                                xTsb[:, 4 * j + c, :],
                                clh[:, c, :], start=(c == 0), stop=(c == 3),
                                skip_group_check=True)
                    if q % 2 == 1:
                        h = q // 2
                        sl = slice(4 * h, 4 * (h + 1))
                        nc.scalar.activation(assn[:, sl, :], p1[:], ACTF.Copy)
                        if h == NH - 1:
                            # Act square: DVE is backlogged at phase-A end
                            with nc.allow_low_precision("fp16 stats sq"):
                                nc.scalar.activation(asq[:, sl, :],
                                                     assn[:, sl, :],
                                                     ACTF.Square)
                        else:
                            nc.vector.tensor_tensor(asq[:, sl, :],
                                                    assn[:, sl, :],
                                                    assn[:, sl, :],
                                                    op=OP.mult)

                def stats(h):
                    for j in range(4):
                        t = 4 * h + j
                        nc.tensor.matmul(pstat[:, 0, :], ones16[:],
                                         asq[:, t, :],
                                         start=(t == 0), stop=(t == nt - 1),
                                         skip_group_check=True)

                sacc = persist.tile([128, KG], F16, name="sacc")

                def ssum(c):
                    # DVE free-axis partial sum of assn over 16 tiles
                    with nc.allow_low_precision("fp16 stats partials"):
                        if c == 0:
                            nc.vector.tensor_reduce(
                                sacc[:],
                                assn[:, :16, :].rearrange("p t k -> p k t"),
                                axis=AX.X, op=OP.add)
                            return
                        red = work.tile([128, KG], F16, name="red", tag="red",
                                        bufs=2)
                        nc.vector.tensor_reduce(
                            red[:],
                            assn[:, 16 * c:16 * (c + 1), :]
                            .rearrange("p t k -> p k t"),
                            axis=AX.X, op=OP.add)
                        nc.vector.tensor_tensor(sacc[:], sacc[:], red[:],
                                                op=OP.add)

                # Stats matmuls are emitted 3 half-groups behind the assn
                # evacuations they read: the PE queue is in-order, so a stats
                # matmul whose Act/DVE evacuation hasn't retired yet would
                # stall the whole PE pipeline.
                stats_done = 0
                ssum_done = 0
                for q in range(NQ + LAG):
                    if q < NQ:
                        produce(q)
                    if q >= LAG:
                        cq = q - LAG
                        consume(cq)
                        ready_h = (cq + 1) // 2 - 3
                        while stats_done < ready_h:
                            stats(stats_done)
                            stats_done += 1
                        while ssum_done < min(3, ready_h // 4):
                            ssum(ssum_done)
                            ssum_done += 1
                while stats_done < NH:
                    stats(stats_done)
                    stats_done += 1
                while ssum_done < 3:
                    ssum(ssum_done)
                    ssum_done += 1
                # Token-sum group in the same bank MUST come strictly after
                # the q-group's stop: start=True clears the whole bank's
                # has_written bits, which would corrupt an in-flight q
                # accumulation. Tiles 48-63 on PE, then the DVE partial
                # (tiles 0-47) via ones@sacc closes the group.
                for t in range(3 * nt // 4, nt):
                    nc.tensor.matmul(pstat[:, 1, :], ones16[:],
                                     assn[:, t, :],
                                     start=(t == 3 * nt // 4), stop=False,
                                     skip_group_check=True)
                nc.tensor.matmul(pstat[:, 1, :], ones16[:], sacc[:],
                                 start=False, stop=True,
                                 skip_group_check=True)

                # ---- neck: stats all-reduce + BN parameters ----
                # stats_sb layout: [sum_sq (q), sum (s)]
                nc.vector.tensor_copy(stats_sb[:], pstat[:])

            if with_collective:
                nc.sync.dma_start(stats_in[:], stats_sb[:])
                nc.gpsimd.collective_compute(
                    "AllReduce", OP.add,
                    replica_groups=[list(range(n_cores))],
                    ins=[stats_in.opt()], outs=[stats_out.opt()])
                nc.sync.dma_start(stats_g[:], stats_out[:])
            else:
                # single-core stand-in for the collective hop
                nc.sync.dma_start(stats_g[:], stats_sb[:])

            t_s2 = work.tile([1, KG], F32, name="t_s2", tag="sv2", bufs=4)
            t_vr = work.tile([1, KG], F32, name="t_vr", tag="sv2", bufs=4)
            t_ln = work.tile([1, KG], F32, name="t_ln", tag="sv2", bufs=4)
            t_sc = work.tile([1, KG], F32, name="t_sc", tag="sv2", bufs=4)
            t_mc = work.tile([1, KG], F32, name="t_mc", tag="sv2", bufs=4)
            inv_n = 1.0 / float(total_tok)
            # var = inv_n*(q - inv_n*s^2); rsqrt via exp(-0.5 ln(.)) with the
            # inv_n factor folded into the Ln's scale operand
            q_row, s_row = stats_g[:, :KG], stats_g[:, KG:]
            nc.vector.tensor_tensor(t_s2[:], s_row, s_row, op=OP.mult)
            nc.vector.scalar_tensor_tensor(t_vr[:], t_s2[:], -inv_n, q_row,
                                           op0=OP.mult, op1=OP.add)
            nc.scalar.activation(t_ln[:], t_vr[:], ACTF.Ln, bias=eps_sb[:],
                                 scale=inv_n)
            nc.scalar.activation(t_sc[:], t_ln[:], ACTF.Exp, scale=-0.5)
            with nc.allow_low_precision("fp16 bn scale"):
                nc.vector.tensor_tensor(ss16[:, :KG], t_sc[:], gamma[:],
                                        op=OP.mult)
            # shift = beta - (inv_n*s)*scale_f32*gamma; use fp16 scale copy
            with nc.allow_low_precision("fp16 bn shift"):
                nc.vector.scalar_tensor_tensor(t_mc[:], s_row, inv_n,
                                               ss16[:, :KG],
                                               op0=OP.mult, op1=OP.mult)
                nc.vector.tensor_tensor(ss16[:, KG:], beta[:], t_mc[:],
                                        op=OP.subtract)
            nc.gpsimd.partition_broadcast(bcB[:], ss16[:])
            scale_b = bcB[:, :KG].rearrange("p (a k) -> p a k", a=1)
            shift_b = bcB[:, KG:].rearrange("p (a k) -> p a k", a=1)

            # ---- phase BC: softmax + vlad + normalize, per batch ----
            with (
                tc.tile_pool(name="psB", bufs=2, space="PSUM") as psB,
                tc.tile_pool(name="elem", bufs=2) as elem,
                tc.tile_pool(name="vpost", bufs=2) as vpost,
            ):
                state = {}
                tebufs = {}

                def te_chunk(t0, n, pool_add=False):
                    # te = exp(scale*assn + shift) for token tiles [t0,t0+n)
                    te = elem.tile([128, n, KG], F16, name="te",
                                   tag=f"te{t0}_{n}", bufs=1)
                    nc.vector.tensor_tensor(
                        te[:], assn[:, t0:t0 + n, :],
                        scale_b.to_broadcast([128, n, KG]), op=OP.mult)
                    eng = nc.gpsimd if pool_add else nc.vector
                    eng.tensor_tensor(
                        te[:], te[:], shift_b.to_broadcast([128, n, KG]),
                        op=OP.add)
                    nc.scalar.activation(te[:], te[:], ACTF.Exp)
                    tebufs[t0] = te

                def sm_chunk(t0, n):
                    # normalize: sm = te / sum_k te, dropping ghosts
                    te = tebufs.pop(t0)
                    # pairwise-add halves at fp16 2x before the 1x reduce
                    dh = work.tile([128, n, KG // 2], F16, name="dh",
                                   tag=f"dh{n}", bufs=3)
                    with nc.allow_low_precision("fp16 softmax denom"):
                        nc.vector.tensor_tensor(dh[:], te[:, :, :KG // 2],
                                                te[:, :, KG // 2:], op=OP.add)
                    denom = work.tile([128, n], F16, name="denom", tag=f"dn{n}",
                                      bufs=3)
                    with nc.allow_low_precision("fp16 softmax denom"):
                        nc.vector.tensor_reduce(denom[:], dh[:], axis=AX.X,
                                                op=OP.add)
                    recip = work.tile([128, n], F16, name="recip", tag=f"rc{n}",
                                      bufs=3)
                    with nc.allow_low_precision("fp16 softmax recip"):
                        nc.vector.reciprocal(recip[:], denom[:])
                    nc.vector.tensor_tensor(
                        sm[:, t0:t0 + n, :], te[:, :, :K],
                        recip[:].rearrange("p (t a) -> p t a", a=1)
                        .to_broadcast([128, n, K]), op=OP.mult)

                # chunking: small first chunks so the first vlad matmuls can
                # start early; full batches later for low op overhead
                CHUNKS = [(0, 1), (1, 1), (2, 2), (4, 4), (8, 8)] + [
                    (b * TPB, TPB) for b in range(1, b_loc - 1)] + [
                    ((b_loc - 1) * TPB, TPB // 2),
                    ((b_loc - 1) * TPB + TPB // 2, TPB // 2)]

                def mm_stage(b):
                    t0 = b * TPB
                    pv = psB.tile([128, 4, K], F32, name="pv", tag="pv")
                    pas = psB.tile([1, K], F32, name="pas", tag="pas")
                    # a_sum first: its PSUM lands while the vlad c-groups
                    # stream, so av is ready before the last c-group stops
                    for i in range(TPB):
                        nc.tensor.matmul(pas[:], ones16[:], sm[:, t0 + i, :],
                                         start=(i == 0), stop=(i == TPB - 1),
                                         skip_group_check=True)
                    pa16 = work.tile([1, K], F16, name="pa16", tag="pa16",
                                     bufs=2)
                    with nc.allow_low_precision("fp16 a_sum"):
                        nc.scalar.activation(pa16[:], pas[:], ACTF.Copy)
                    av = vpost.tile([128, 4, K], F16, name="av", tag="av")
                    if b == b_loc - 1:
                        # last batch: broadcast via PE + DVE to skip the Pool
                        # round-trips on the tail-critical path
                        pamP = psB.tile([128, K], F32, name="pamP", tag="pamP")
                        nc.tensor.matmul(pamP[:], ones_row[:], pa16[:],
                                         start=True, stop=True,
                                         skip_group_check=True)
                        nc.vector.tensor_tensor(
                            av[:], c2n[:],
                            pamP[:].rearrange("p (a k) -> p a k", a=1)
                            .to_broadcast([128, 4, K]), op=OP.mult)
                    else:
                        pamB = vpost.tile([128, K], F16, name="pamB",
                                          tag="pam")
                        nc.gpsimd.partition_broadcast(pamB[:], pa16[:])
                        nc.gpsimd.tensor_tensor(
                            av[:], c2n[:],
                            pamB[:].rearrange("p (a k) -> p a k", a=1)
                            .to_broadcast([128, 4, K]), op=OP.mult)
                    # vlad: x stationary with d = 4p + c column layout
                    for c in range(4):
                        for i in range(TPB):
                            t = t0 + i
                            nc.tensor.matmul(
                                pv[:, c, :],
                                xh[:, t, c::4],
                                sm[:, t, :],
                                start=(i == 0), stop=(i == TPB - 1),
                                skip_group_check=True)
                    state[b] = (pv, av)

                def post_stage(b):
                    pv, av = state.pop(b)
                    v = vpost.tile([128, 4, K], F16, name="v", tag="v")
                    sq = vpost.tile([128, 4, K], F16, name="sq", tag="sq")
                    pnrm = psB.tile([1, K], F32, name="pnrm", tag="pnrm")
                    # halves over the c dim: v/sq/pnrm for c<2 overlap the
                    # c2/c3 vlad matmuls of this batch
                    for hc in range(2):
                        cs = slice(2 * hc, 2 * hc + 2)
                        with nc.allow_low_precision("fp16 vlad residual"):
                            nc.vector.tensor_tensor(v[:, cs, :], pv[:, cs, :],
                                                    av[:, cs, :],
                                                    op=OP.subtract)
                        with nc.allow_low_precision("fp16 norm squares"):
                            nc.scalar.activation(sq[:, cs, :], v[:, cs, :],
                                                 ACTF.Square)
                        for c in range(2 * hc, 2 * hc + 2):
                            nc.tensor.matmul(pnrm[:], ones16[:], sq[:, c, :],
                                             start=(c == 0), stop=(c == 3),
                                             skip_group_check=True)
                    # y = v * rsqrt(64*nrm2): intra-norm and global L2 norm
                    # folded (flat norm is exactly sqrt(64) post intra-norm)
                    rnl = work.tile([1, K], F32, name="rnl", tag="rnl")
                    nc.scalar.activation(rnl[:], pnrm[:], ACTF.Ln, scale=64.0)
                    rn16 = work.tile([1, K], F16, name="rn16", tag="rn16")
                    with nc.allow_low_precision("fp16 norm scale"):
                        nc.scalar.activation(rn16[:], rnl[:], ACTF.Exp,
                                             scale=-0.5)
                    vf = vpost.tile([128, 4, K], F32, name="vf", tag="vf")
                    yb = y[b, :].rearrange("(p c k) -> p c k", p=128, k=K)
                    if b == b_loc - 1:
                        # last batch: PE broadcast + DVE scale + split y
                        # write to shorten the tail-critical chain
                        prnP = psB.tile([128, K], F32, name="prnP", tag="pamP")
                        nc.tensor.matmul(prnP[:], ones_row[:], rn16[:],
                                         start=True, stop=True,
                                         skip_group_check=True)
                        prnPv = prnP[:].rearrange("p (a k) -> p a k", a=1)
                        for hc in range(2):
                            cs = slice(2 * hc, 2 * hc + 2)
                            nc.vector.tensor_tensor(
                                vf[:, cs, :], v[:, cs, :],
                                prnPv.to_broadcast([128, 2, K]), op=OP.mult)
                            nc.sync.dma_start(yb[:, cs, :], vf[:, cs, :])
                    else:
                        prnB = vpost.tile([128, K], F16, name="prnB",
                                          tag="prn")
                        nc.gpsimd.partition_broadcast(prnB[:], rn16[:])
                        prnBv = prnB[:].rearrange("p (a k) -> p a k", a=1)
                        nc.gpsimd.tensor_tensor(
                            vf[:], v[:], prnBv.to_broadcast([128, 4, K]),
                            op=OP.mult)
                        nc.sync.dma_start(yb[:, :, :], vf[:])

                # Skew-by-one software pipeline: each sm chunk is emitted one
                # te-chunk later so the DVE stream never waits on an Act exp;
                # vlad (PE) and post stages weave in as batches complete.
                nch = len(CHUNKS)
                done_b = 0
                for i in range(nch + 1):
                    if i < nch:
                        t0, n = CHUNKS[i]
                        te_chunk(t0, n, pool_add=False)
                    if i >= 1:
                        t0, n = CHUNKS[i - 1]
                        sm_chunk(t0, n)
                        if (t0 + n) % TPB == 0:     # batch done_b fully sm'd
                            mm_stage(done_b)
                            if done_b >= 1:
                                post_stage(done_b - 1)
                            done_b += 1
                post_stage(b_loc - 1)
    nc.compile()
    return nc


_CACHE = {}


def _get(b_loc, n_cores, with_collective):
    key = (b_loc, n_cores, with_collective)
    if key not in _CACHE:
        _CACHE[key] = build(b_loc, n_cores, with_collective)
    return _CACHE[key]


def make_in_maps(x, clusters, clusters2, bn_gamma, bn_beta, n_cores=N_CORES):
    B = x.shape[0]
    b_loc = B // n_cores
    shared = {
        "clusters": np.ascontiguousarray(clusters, np.float32),
        "clusters2": np.ascontiguousarray(
            np.asarray(clusters2).reshape(D, K), np.float32),
        "bn_gamma": np.ascontiguousarray(
            np.asarray(bn_gamma).reshape(1, KG), np.float32),
        "bn_beta": np.ascontiguousarray(
            np.asarray(bn_beta).reshape(1, KG), np.float32),
    }
    in_maps = []
    for i in range(n_cores):
        m = dict(shared)
        m["x"] = np.ascontiguousarray(
            np.asarray(x[i * b_loc:(i + 1) * b_loc]).reshape(
                b_loc * N_SEQ, D), np.float32)
        in_maps.append(m)
    return in_maps


def kernel(x, clusters, clusters2, bn_gamma, bn_beta):
    B, N, Dd = x.shape
    assert (N, Dd) == (N_SEQ, D) and B % N_CORES == 0
    b_loc = B // N_CORES
    nc = _get(b_loc, N_CORES, True)
    in_maps = make_in_maps(x, clusters, clusters2, bn_gamma, bn_beta)
    res = run_bass_kernel_spmd(nc, in_maps, core_ids=list(range(N_CORES)))
    out = np.concatenate([res.results[i]["y"] for i in range(N_CORES)], axis=0)
    return out


# revision 53
# speedup vs baseline: 2.0667x; 1.0190x over previous
"""NetVLAD-style vq_codebook kernel for 8 Trainium2 NeuronCores.

Reference computation (per full input):
  assn = BN(x @ clusters); softmax over 80 clusters, drop 16 ghosts
  vlad[b,d,k] = sum_n assn[b,n,k] x[b,n,d] - a_sum[b,k]*clusters2[d,k]
  intra-normalize over d, flatten, global L2 normalize -> (B, D*K)

Sharding: data-parallel over batch B (B/8 batches per core). BatchNorm
statistics (sum and sum-of-squares per cluster column) are all-reduced
across the 8 cores (2*80 floats). Everything else is local.

Key structure (v2, redesigned around the engine cost model):
 - x cast-loaded fp32->fp16 by SWDGE DMA in token-partition layout.
 - x^T (d-partition) via PE transposes (is_transpose matmuls writing
   fp16 PSUM), software-pipelined at 2-tile granularity and batch-
   evacuated to SBUF by DVE and Act in a ~17:15 split.
 - assignment matmul per token tile: 4 accumulating (128x128)@(128x80)
   fp16 matmuls. BN sum-of-squares via a long PE ones-matmul group;
   BN sums via DVE free-axis reduces + a PE partition reduce, sharing
   one PSUM bank with strictly sequential accumulation groups.
 - softmax: scale/shift as fp16 2x DVE tensor-tensor ops, Exp on Act
   (one activation table for ln/exp/copy/square -> a single load),
   pairwise-halved fp16 denominator, 1/sqrt as exp(-0.5*ln(x)).
 - vlad with x stationary in a d=4p+c column layout so the final DMA
   writes 1KB-contiguous runs; a_sum accumulated directly as [1,64]
   before the vlad groups so a_sum*clusters2 overlaps them.
 - global L2 norm folded analytically: after intra-normalization the
   flat norm is exactly sqrt(64), so y = v * rsqrt(64*nrm2[k]).
 - one serial neck (stats hop + BN math) between the assignment pass
   and the softmax/vlad pass; batch-0 softmax runs in small chunks so
   the first vlad matmuls start early.
"""

import sys

for _p in ("/opt/trn_rl_repo", "/root/.axon_site/_ro/trn_rl_repo"):
    if _p not in sys.path:
        sys.path.insert(0, _p)

import numpy as np

import concourse.bacc as bacc
import concourse.mybir as mybir
import concourse.tile as tile
from concourse.bass_utils import run_bass_kernel_spmd

F32 = mybir.dt.float32
F16 = mybir.dt.float16
AX = mybir.AxisListType
OP = mybir.AluOpType
ACTF = mybir.ActivationFunctionType

N_CORES = 8
D = 512
KG = 80          # clusters + ghosts
K = 64           # real clusters
N_SEQ = 2048
TPB = N_SEQ // 128   # token tiles per batch = 16
BN_EPS = 1e-5

# Tunables
import os as _os
XBAR_QUARTERS = tuple(
    int(v) for v in _os.environ.get("K_XBAR", "").split(",") if v)
LAG = int(_os.environ.get("K_LAG", "5"))
LOADS = tuple(int(v) for v in _os.environ.get("K_LOADS", "4,4").split(","))
PXT_BUFS = int(_os.environ.get("K_PXT", "4"))


def build(b_loc=4, n_cores=N_CORES, with_collective=True):
    """Build the per-core program. b_loc = batches per core."""
    nt = b_loc * TPB                # token tiles per core
    tok = nt * 128                  # tokens per core
    total_tok = tok * n_cores       # global token count for BN stats
    NH = nt // 4                    # half-groups (4 tiles each)

    nc = bacc.Bacc("TRN2", target_bir_lowering=False, debug=False,
                   dynamic_dma_scratch_size=65536)

    x = nc.declare_dram_parameter("x", [tok, D], F32, isOutput=False)
    cl = nc.declare_dram_parameter("clusters", [D, KG], F32, isOutput=False)
    c2 = nc.declare_dram_parameter("clusters2", [D, K], F32, isOutput=False)
    gam = nc.declare_dram_parameter("bn_gamma", [1, KG], F32, isOutput=False)
    bet = nc.declare_dram_parameter("bn_beta", [1, KG], F32, isOutput=False)
    y = nc.declare_dram_parameter("y", [b_loc, D * K], F32, isOutput=True)

    eye_c = nc.inline_tensor(np.eye(128, dtype=np.float16), name="c_eye")

    with tile.TileContext(nc) as tc:
        with (
            tc.tile_pool(name="persist", bufs=1) as persist,
            tc.tile_pool(name="work", bufs=4) as work,
            tc.tile_pool(name="dram", bufs=1, space="DRAM") as dram,
        ):
            # ---- persistent SBUF tensors ----
            xh = persist.tile([128, nt, D], F16, name="xh")
            assn = persist.tile([128, nt, KG], F16, name="assn")
            asq = persist.tile([128, nt, KG], F16, name="asq")
            sm = persist.tile([128, nt, K], F16, name="sm")
            idn = persist.tile([128, 128], F16, name="idn")
            clh = persist.tile([128, 4, KG], F16, name="clh")
            c2n = persist.tile([128, 4, K], F16, name="c2n")
            ones16 = persist.tile([128, 1], F16, name="ones16")
            ones_row = persist.tile([1, 128], F16, name="ones_row")
            gamma = persist.tile([1, KG], F32, name="gamma")
            beta = persist.tile([1, KG], F32, name="beta")
            ss16 = persist.tile([1, 2 * KG], F16, name="ss16")
            bcB = persist.tile([128, 2 * KG], F16, name="bcB")
            stats_sb = persist.tile([1, 2 * KG], F32, name="stats_sb")
            stats_g = persist.tile([1, 2 * KG], F32, name="stats_g")
            actwarm = persist.tile([1, 1], F32, name="actwarm")
            eps_sb = persist.tile([1, 1], F32, name="eps_sb")

            stats_in = dram.tile([1, 2 * KG], F32, name="stats_in")
            stats_out = dram.tile([1, 2 * KG], F32, name="stats_out")

            # ---- phase 0: constants + x load/cast ----
            nc.sync.dma_start(gamma[:], gam[:, :])
            nc.sync.dma_start(beta[:], bet[:, :])
            nc.sync.dma_start(idn[:], eye_c.ap()[:, :])
            nc.vector.memset(ones16[:], 1.0)
            nc.vector.memset(ones_row[:], 1.0)
            nc.vector.memset(eps_sb[:], BN_EPS)
            # Pre-load the one activation table covering every function this
            # kernel uses (ln/exp/copy/square), so the table-load inserter
            # doesn't alternate between ln-only and exp-only sets. Best
            # effort: fall back to automatic insertion if the set is absent.
            try:
                from concourse.hw_specs import get_activation_tables
                tabs = get_activation_tables(nc.m.arch)
                set_id = list(tabs).index("natural_log_exp_and_others")
                nc.scalar.add_instruction(mybir.InstLoadActFuncSet(
                    name=nc.get_next_instruction_name(),
                    engine=mybir.EngineType.Activation,
                    act_func_set_id=set_id, ins=[], outs=[]))
            except (ImportError, ValueError, KeyError):
                pass
            # Touch the activation engine early so any residual table load
            # happens off the critical path.
            nc.scalar.activation(actwarm[:], gamma[:, :1], ACTF.Ln)

            # x cast-DMA (SWDGE casts fp32->fp16 in the DMA engines; HBM
            # read is the real cost). Small first chunks start the PE
            # transpose pipeline sooner.
            xr = x.ap().rearrange("(t p) d -> p t d", p=128)
            t0 = 0
            for sz in LOADS + (8,) * ((nt - sum(LOADS)) // 8):
                nc.gpsimd.dma_start(
                    xh[:, t0:t0 + sz, :], xr[:, t0:t0 + sz, :])
                t0 += sz
            assert t0 == nt
            # clusters via HWDGE (fp32) + DVE cast: the Pool/SWDGE queue is
            # saturated by the x loads, and clh is needed early.
            clf = work.tile([128, 4, KG], F32, name="clf", tag="clf", bufs=1)
            nc.sync.dma_start(
                clf[:], cl.ap().rearrange("(c p) k -> p c k", p=128))
            nc.vector.tensor_copy(clh[:], clf[:])
            # clusters2 in d=4p+c layout (matches vlad output partitioning);
            # not needed until the post stage, so SWDGE order is fine.
            nc.gpsimd.dma_start(
                c2n[:], c2.ap().rearrange("(p c) k -> p c k", c=4))

            # ---- phase A: transposes + assignment matmul + BN stats ----
            with tc.tile_pool(name="psA", bufs=2, space="PSUM") as psA:
                # separate banks so the token-sum group can run while the
                # sum-of-squares group is still accumulating (start=True
                # clears a whole bank's has_written bits)
                pstat_q = psA.tile([1, KG], F32, name="pstat_q",
                                   tag="st_q", bufs=1)
                pstat_s = psA.tile([1, KG], F32, name="pstat_s",
                                   tag="st_s", bufs=1)

                NQ = nt // 2            # quarter-groups (2 tiles each)
                xtbufs = {}
                p1bufs = {}

                def produce(q):
                    # xTsb for quarter q: [128, 8, 128] fp16 with block
                    # e = 4j + c holding x[tile 2q+j, 128c:128c+128]^T
                    xTsb = work.tile([128, 8, 128], F16, name=f"xT{q}",
                                     tag="xt", bufs=LAG + 2)
                    if q in XBAR_QUARTERS:
                        nc.sync.dma_start(xTsb[:, :, :],
                                          xh[:, 2 * q:2 * (q + 1), :],
                                          transpose=True)
                    else:
                        pxt = psA.tile([128, 8, 128], F16, name="pxt",
                                       tag="pxt", bufs=PXT_BUFS)
                        for j in range(2):
                            t = 2 * q + j
                            for c in range(4):
                                nc.tensor.transpose(
                                    pxt[:, 4 * j + c, :],
                                    xh[:, t, 128 * c:128 * (c + 1)], idn[:])
                        # batched PSUM->SBUF evacuation; split ~17:15
                        # between DVE and Act (DVE is ~1.5x faster per elem)
                        if (q * 17) // 32 != ((q - 1) * 17) // 32:
                            nc.vector.tensor_copy(xTsb[:], pxt[:])
                        else:
                            nc.scalar.activation(xTsb[:], pxt[:], ACTF.Copy)
                    xtbufs[q] = xTsb

                def consume(q):
                    xTsb = xtbufs.pop(q)
                    if q % 2 == 0:
                        p1bufs[q // 2] = psA.tile([128, 4, KG], F32,
                                                  name="p1", tag="p1", bufs=2)
                    p1 = p1bufs[q // 2]
                    for j in range(2):
                        for c in range(4):
                            nc.tensor.matmul(
                                p1[:, 2 * (q % 2) + j, :],
                                xTsb[:, 4 * j + c, :],
                                clh[:, c, :], start=(c == 0), stop=(c == 3),
                                skip_group_check=True)
                    if q % 2 == 1:
                        h = q // 2
                        sl = slice(4 * h, 4 * (h + 1))
                        nc.scalar.activation(assn[:, sl, :], p1[:], ACTF.Copy)
                        if h == NH - 1:
                            # Act square: DVE is backlogged at phase-A end
                            with nc.allow_low_precision("fp16 stats sq"):
                                nc.scalar.activation(asq[:, sl, :],
                                                     assn[:, sl, :],
                                                     ACTF.Square)
                        else:
                            nc.vector.tensor_tensor(asq[:, sl, :],
                                                    assn[:, sl, :],
                                                    assn[:, sl, :],
                                                    op=OP.mult)

                def stats(h):
                    for j in range(4):
                        t = 4 * h + j
                        nc.tensor.matmul(pstat_q[:], ones16[:],
                                         asq[:, t, :],
                                         start=(t == 0), stop=(t == nt - 1),
                                         skip_group_check=True)
                        if t >= 3 * nt // 4:
                            nc.tensor.matmul(pstat_s[:], ones16[:],
                                             assn[:, t, :],
                                             start=(t == 3 * nt // 4),
                                             stop=False,
                                             skip_group_check=True)

                sacc = persist.tile([128, KG], F16, name="sacc")

                def ssum(c):
                    # DVE free-axis partial sum of assn over 16 tiles
                    with nc.allow_low_precision("fp16 stats partials"):
                        if c == 0:
                            nc.vector.tensor_reduce(
                                sacc[:],
                                assn[:, :16, :].rearrange("p t k -> p k t"),
                                axis=AX.X, op=OP.add)
                            return
                        red = work.tile([128, KG], F16, name="red", tag="red",
                                        bufs=2)
                        nc.vector.tensor_reduce(
                            red[:],
                            assn[:, 16 * c:16 * (c + 1), :]
                            .rearrange("p t k -> p k t"),
                            axis=AX.X, op=OP.add)
                        nc.vector.tensor_tensor(sacc[:], sacc[:], red[:],
                                                op=OP.add)

                # Stats matmuls are emitted 3 half-groups behind the assn
                # evacuations they read: the PE queue is in-order, so a stats
                # matmul whose Act/DVE evacuation hasn't retired yet would
                # stall the whole PE pipeline.
                stats_done = 0
                ssum_done = 0
                for q in range(NQ + LAG):
                    if q < NQ:
                        produce(q)
                    if q >= LAG:
                        cq = q - LAG
                        consume(cq)
                        ready_h = (cq + 1) // 2 - 3
                        while stats_done < ready_h:
                            stats(stats_done)
                            stats_done += 1
                        while ssum_done < min(3, ready_h // 4):
                            ssum(ssum_done)
                            ssum_done += 1
                while stats_done < NH:
                    stats(stats_done)
                    stats_done += 1
                while ssum_done < 3:
                    ssum(ssum_done)
                    ssum_done += 1
                # close the token-sum group with the DVE partial (tiles 0-47)
                nc.tensor.matmul(pstat_s[:], ones16[:], sacc[:],
                                 start=False, stop=True,
                                 skip_group_check=True)

                # ---- neck: stats all-reduce + BN parameters ----
                # stats_sb layout: [sum_sq (q), sum (s)]
                nc.vector.tensor_copy(stats_sb[:, :KG], pstat_q[:])
                nc.vector.tensor_copy(stats_sb[:, KG:], pstat_s[:])

            if with_collective:
                nc.sync.dma_start(stats_in[:], stats_sb[:])
                nc.gpsimd.collective_compute(
                    "AllReduce", OP.add,
                    replica_groups=[list(range(n_cores))],
                    ins=[stats_in.opt()], outs=[stats_out.opt()])
                nc.sync.dma_start(stats_g[:], stats_out[:])
            else:
                # single-core stand-in for the collective hop
                nc.sync.dma_start(stats_g[:], stats_sb[:])

            t_s2 = work.tile([1, KG], F32, name="t_s2", tag="sv2", bufs=4)
            t_vr = work.tile([1, KG], F32, name="t_vr", tag="sv2", bufs=4)
            t_ln = work.tile([1, KG], F32, name="t_ln", tag="sv2", bufs=4)
            t_sc = work.tile([1, KG], F32, name="t_sc", tag="sv2", bufs=4)
            t_mc = work.tile([1, KG], F32, name="t_mc", tag="sv2", bufs=4)
            inv_n = 1.0 / float(total_tok)
            # var = inv_n*(q - inv_n*s^2); rsqrt via exp(-0.5 ln(.)) with the
            # inv_n factor folded into the Ln's scale operand
            q_row, s_row = stats_g[:, :KG], stats_g[:, KG:]
            nc.vector.tensor_tensor(t_s2[:], s_row, s_row, op=OP.mult)
            nc.vector.scalar_tensor_tensor(t_vr[:], t_s2[:], -inv_n, q_row,
                                           op0=OP.mult, op1=OP.add)
            nc.scalar.activation(t_ln[:], t_vr[:], ACTF.Ln, bias=eps_sb[:],
                                 scale=inv_n)
            nc.scalar.activation(t_sc[:], t_ln[:], ACTF.Exp, scale=-0.5)
            with nc.allow_low_precision("fp16 bn scale"):
                nc.vector.tensor_tensor(ss16[:, :KG], t_sc[:], gamma[:],
                                        op=OP.mult)
            # shift = beta - (inv_n*s)*scale_f32*gamma; use fp16 scale copy
            with nc.allow_low_precision("fp16 bn shift"):
                nc.vector.scalar_tensor_tensor(t_mc[:], s_row, inv_n,
                                               ss16[:, :KG],
                                               op0=OP.mult, op1=OP.mult)
                nc.vector.tensor_tensor(ss16[:, KG:], beta[:], t_mc[:],
                                        op=OP.subtract)
            nc.gpsimd.partition_broadcast(bcB[:], ss16[:])
            scale_b = bcB[:, :KG].rearrange("p (a k) -> p a k", a=1)
            shift_b = bcB[:, KG:].rearrange("p (a k) -> p a k", a=1)

            # ---- phase BC: softmax + vlad + normalize, per batch ----
            with (
                tc.tile_pool(name="psB", bufs=2, space="PSUM") as psB,
                tc.tile_pool(name="elem", bufs=2) as elem,
                tc.tile_pool(name="vpost", bufs=2) as vpost,
            ):
                state = {}
                tebufs = {}

                def te_chunk(t0, n, pool_add=False):
                    # te = exp(scale*assn + shift) for token tiles [t0,t0+n)
                    te = elem.tile([128, n, KG], F16, name="te",
                                   tag=f"te{t0}_{n}", bufs=1)
                    nc.vector.tensor_tensor(
                        te[:], assn[:, t0:t0 + n, :],
                        scale_b.to_broadcast([128, n, KG]), op=OP.mult)
                    eng = nc.gpsimd if pool_add else nc.vector
                    eng.tensor_tensor(
                        te[:], te[:], shift_b.to_broadcast([128, n, KG]),
                        op=OP.add)
                    nc.scalar.activation(te[:], te[:], ACTF.Exp)
                    tebufs[t0] = te

                def sm_chunk(t0, n):
                    # normalize: sm = te / sum_k te, dropping ghosts
                    te = tebufs.pop(t0)
                    # pairwise-add tree at fp16 2x before the 1x reduce
                    dh = work.tile([128, n, KG // 2], F16, name="dh",
                                   tag=f"dh{n}", bufs=3)
                    dh2 = work.tile([128, n, KG // 4], F16, name="dh2",
                                    tag=f"dh2{n}", bufs=3)
                    with nc.allow_low_precision("fp16 softmax denom"):
                        nc.vector.tensor_tensor(dh[:], te[:, :, :KG // 2],
                                                te[:, :, KG // 2:], op=OP.add)
                        nc.vector.tensor_tensor(dh2[:], dh[:, :, :KG // 4],
                                                dh[:, :, KG // 4:], op=OP.add)
                    denom = work.tile([128, n], F16, name="denom", tag=f"dn{n}",
                                      bufs=3)
                    with nc.allow_low_precision("fp16 softmax denom"):
                        nc.vector.tensor_reduce(denom[:], dh2[:], axis=AX.X,
                                                op=OP.add)
                    recip = work.tile([128, n], F16, name="recip", tag=f"rc{n}",
                                      bufs=3)
                    with nc.allow_low_precision("fp16 softmax recip"):
                        nc.vector.reciprocal(recip[:], denom[:])
                    nc.vector.tensor_tensor(
                        sm[:, t0:t0 + n, :], te[:, :, :K],
                        recip[:].rearrange("p (t a) -> p t a", a=1)
                        .to_broadcast([128, n, K]), op=OP.mult)

                # chunking: small first chunks so the first vlad matmuls can
                # start early; full batches later for low op overhead
                CHUNKS = [(0, 1), (1, 1), (2, 2), (4, 4), (8, 8)] + [
                    (b * TPB, TPB) for b in range(1, b_loc - 1)] + [
                    ((b_loc - 1) * TPB, TPB // 2),
                    ((b_loc - 1) * TPB + TPB // 2, TPB // 2)]

                def mm_stage(b):
                    t0 = b * TPB
                    pv = psB.tile([128, 4, K], F32, name="pv", tag="pv")
                    pas = psB.tile([1, K], F32, name="pas", tag="pas")
                    # a_sum first: its PSUM lands while the vlad c-groups
                    # stream, so av is ready before the last c-group stops
                    for i in range(TPB):
                        nc.tensor.matmul(pas[:], ones16[:], sm[:, t0 + i, :],
                                         start=(i == 0), stop=(i == TPB - 1),
                                         skip_group_check=True)
                    pa16 = work.tile([1, K], F16, name="pa16", tag="pa16",
                                     bufs=2)
                    with nc.allow_low_precision("fp16 a_sum"):
                        nc.scalar.activation(pa16[:], pas[:], ACTF.Copy)
                    av = vpost.tile([128, 4, K], F16, name="av", tag="av")
                    if b == b_loc - 1:
                        # last batch: broadcast via PE + DVE to skip the Pool
                        # round-trips on the tail-critical path
                        pamP = psB.tile([128, K], F32, name="pamP", tag="pamP")
                        nc.tensor.matmul(pamP[:], ones_row[:], pa16[:],
                                         start=True, stop=True,
                                         skip_group_check=True)
                        nc.vector.tensor_tensor(
                            av[:], c2n[:],
                            pamP[:].rearrange("p (a k) -> p a k", a=1)
                            .to_broadcast([128, 4, K]), op=OP.mult)
                    else:
                        pamB = vpost.tile([128, K], F16, name="pamB",
                                          tag="pam")
                        nc.gpsimd.partition_broadcast(pamB[:], pa16[:])
                        nc.gpsimd.tensor_tensor(
                            av[:], c2n[:],
                            pamB[:].rearrange("p (a k) -> p a k", a=1)
                            .to_broadcast([128, 4, K]), op=OP.mult)
                    # vlad: x stationary with d = 4p + c column layout
                    for c in range(4):
                        for i in range(TPB):
                            t = t0 + i
                            nc.tensor.matmul(
                                pv[:, c, :],
                                xh[:, t, c::4],
                                sm[:, t, :],
                                start=(i == 0), stop=(i == TPB - 1),
                                skip_group_check=True)
                    state[b] = (pv, av)

                def post_stage(b):
                    pv, av = state.pop(b)
                    v = vpost.tile([128, 4, K], F16, name="v", tag="v")
                    sq = vpost.tile([128, 4, K], F16, name="sq", tag="sq")
                    pnrm = psB.tile([1, K], F32, name="pnrm", tag="pnrm")
                    # halves over the c dim: v/sq/pnrm for c<2 overlap the
                    # c2/c3 vlad matmuls of this batch
                    for hc in range(2):
                        cs = slice(2 * hc, 2 * hc + 2)
                        with nc.allow_low_precision("fp16 vlad residual"):
                            nc.vector.tensor_tensor(v[:, cs, :], pv[:, cs, :],
                                                    av[:, cs, :],
                                                    op=OP.subtract)
                        with nc.allow_low_precision("fp16 norm squares"):
                            nc.scalar.activation(sq[:, cs, :], v[:, cs, :],
                                                 ACTF.Square)
                        for c in range(2 * hc, 2 * hc + 2):
                            nc.tensor.matmul(pnrm[:], ones16[:], sq[:, c, :],
                                             start=(c == 0), stop=(c == 3),
                                             skip_group_check=True)
                    # y = v * rsqrt(64*nrm2): intra-norm and global L2 norm
                    # folded (flat norm is exactly sqrt(64) post intra-norm)
                    rnl = work.tile([1, K], F32, name="rnl", tag="rnl")
                    nc.scalar.activation(rnl[:], pnrm[:], ACTF.Ln, scale=64.0)
                    rn16 = work.tile([1, K], F16, name="rn16", tag="rn16")
                    with nc.allow_low_precision("fp16 norm scale"):
                        nc.scalar.activation(rn16[:], rnl[:], ACTF.Exp,
                                             scale=-0.5)
                    vf = vpost.tile([128, 4, K], F32, name="vf", tag="vf")
                    yb = y[b, :].rearrange("(p c k) -> p c k", p=128, k=K)
                    if b == b_loc - 1:
                        # last batch: PE broadcast + DVE scale + split y
                        # write to shorten the tail-critical chain
                        prnP = psB.tile([128, K], F32, name="prnP", tag="pamP")
                        nc.tensor.matmul(prnP[:], ones_row[:], rn16[:],
                                         start=True, stop=True,
                                         skip_group_check=True)
                        prnPv = prnP[:].rearrange("p (a k) -> p a k", a=1)
                        for hc in range(2):
                            cs = slice(2 * hc, 2 * hc + 2)
                            nc.vector.tensor_tensor(
                                vf[:, cs, :], v[:, cs, :],
                                prnPv.to_broadcast([128, 2, K]), op=OP.mult)
                            nc.sync.dma_start(yb[:, cs, :], vf[:, cs, :])
                    else:
                        prnB = vpost.tile([128, K], F16, name="prnB",
                                          tag="prn")
                        nc.gpsimd.partition_broadcast(prnB[:], rn16[:])
                        prnBv = prnB[:].rearrange("p (a k) -> p a k", a=1)
                        nc.gpsimd.tensor_tensor(
                            vf[:], v[:], prnBv.to_broadcast([128, 4, K]),
                            op=OP.mult)
                        nc.sync.dma_start(yb[:, :, :], vf[:])

                # Skew-by-one software pipeline: each sm chunk is emitted one
                # te-chunk later so the DVE stream never waits on an Act exp;
                # vlad (PE) and post stages weave in as batches complete.
                nch = len(CHUNKS)
                done_b = 0
                for i in range(nch + 1):
                    if i < nch:
                        t0, n = CHUNKS[i]
                        te_chunk(t0, n, pool_add=False)
                    if i >= 1:
                        t0, n = CHUNKS[i - 1]
                        sm_chunk(t0, n)
                        if (t0 + n) % TPB == 0:     # batch done_b fully sm'd
                            mm_stage(done_b)
                            if done_b >= 1:
                                post_stage(done_b - 1)
                            done_b += 1
                post_stage(b_loc - 1)
    nc.compile()
    return nc


_CACHE = {}


def _get(b_loc, n_cores, with_collective):
    key = (b_loc, n_cores, with_collective)
    if key not in _CACHE:
        _CACHE[key] = build(b_loc, n_cores, with_collective)
    return _CACHE[key]


def make_in_maps(x, clusters, clusters2, bn_gamma, bn_beta, n_cores=N_CORES):
    B = x.shape[0]
    b_loc = B // n_cores
    shared = {
        "clusters": np.ascontiguousarray(clusters, np.float32),
        "clusters2": np.ascontiguousarray(
            np.asarray(clusters2).reshape(D, K), np.float32),
        "bn_gamma": np.ascontiguousarray(
            np.asarray(bn_gamma).reshape(1, KG), np.float32),
        "bn_beta": np.ascontiguousarray(
            np.asarray(bn_beta).reshape(1, KG), np.float32),
    }
    in_maps = []
    for i in range(n_cores):
        m = dict(shared)
        m["x"] = np.ascontiguousarray(
            np.asarray(x[i * b_loc:(i + 1) * b_loc]).reshape(
                b_loc * N_SEQ, D), np.float32)
        in_maps.append(m)
    return in_maps


def kernel(x, clusters, clusters2, bn_gamma, bn_beta):
    B, N, Dd = x.shape
    assert (N, Dd) == (N_SEQ, D) and B % N_CORES == 0
    b_loc = B // N_CORES
    nc = _get(b_loc, N_CORES, True)
    in_maps = make_in_maps(x, clusters, clusters2, bn_gamma, bn_beta)
    res = run_bass_kernel_spmd(nc, in_maps, core_ids=list(range(N_CORES)))
    out = np.concatenate([res.results[i]["y"] for i in range(N_CORES)], axis=0)
    return out


# revision 54
# speedup vs baseline: 2.0767x; 1.0049x over previous
"""NetVLAD-style vq_codebook kernel for 8 Trainium2 NeuronCores.

Reference computation (per full input):
  assn = BN(x @ clusters); softmax over 80 clusters, drop 16 ghosts
  vlad[b,d,k] = sum_n assn[b,n,k] x[b,n,d] - a_sum[b,k]*clusters2[d,k]
  intra-normalize over d, flatten, global L2 normalize -> (B, D*K)

Sharding: data-parallel over batch B (B/8 batches per core). BatchNorm
statistics (sum and sum-of-squares per cluster column) are all-reduced
across the 8 cores (2*80 floats). Everything else is local.

Key structure (v2, redesigned around the engine cost model):
 - x cast-loaded fp32->fp16 by SWDGE DMA in token-partition layout.
 - x^T (d-partition) via PE transposes (is_transpose matmuls writing
   fp16 PSUM), software-pipelined at 2-tile granularity and batch-
   evacuated to SBUF by DVE and Act in a ~17:15 split.
 - assignment matmul per token tile: 4 accumulating (128x128)@(128x80)
   fp16 matmuls. BN sum-of-squares via a long PE ones-matmul group;
   BN sums via DVE free-axis reduces + a PE partition reduce, sharing
   one PSUM bank with strictly sequential accumulation groups.
 - softmax: scale/shift as fp16 2x DVE tensor-tensor ops, Exp on Act
   (one activation table for ln/exp/copy/square -> a single load),
   pairwise-halved fp16 denominator, 1/sqrt as exp(-0.5*ln(x)).
 - vlad with x stationary in a d=4p+c column layout so the final DMA
   writes 1KB-contiguous runs; a_sum accumulated directly as [1,64]
   before the vlad groups so a_sum*clusters2 overlaps them.
 - global L2 norm folded analytically: after intra-normalization the
   flat norm is exactly sqrt(64), so y = v * rsqrt(64*nrm2[k]).
 - one serial neck (stats hop + BN math) between the assignment pass
   and the softmax/vlad pass; batch-0 softmax runs in small chunks so
   the first vlad matmuls start early.
"""

import sys

for _p in ("/opt/trn_rl_repo", "/root/.axon_site/_ro/trn_rl_repo"):
    if _p not in sys.path:
        sys.path.insert(0, _p)

import numpy as np

import concourse.bacc as bacc
import concourse.mybir as mybir
import concourse.tile as tile
from concourse.bass_utils import run_bass_kernel_spmd

F32 = mybir.dt.float32
F16 = mybir.dt.float16
AX = mybir.AxisListType
OP = mybir.AluOpType
ACTF = mybir.ActivationFunctionType

N_CORES = 8
D = 512
KG = 80          # clusters + ghosts
K = 64           # real clusters
N_SEQ = 2048
TPB = N_SEQ // 128   # token tiles per batch = 16
BN_EPS = 1e-5

# Tunables
import os as _os
XBAR_QUARTERS = tuple(
    int(v) for v in _os.environ.get("K_XBAR", "").split(",") if v)
LAG = int(_os.environ.get("K_LAG", "5"))
LOADS = tuple(int(v) for v in _os.environ.get("K_LOADS", "4,4").split(","))
PXT_BUFS = int(_os.environ.get("K_PXT", "4"))


def build(b_loc=4, n_cores=N_CORES, with_collective=True):
    """Build the per-core program. b_loc = batches per core."""
    nt = b_loc * TPB                # token tiles per core
    tok = nt * 128                  # tokens per core
    total_tok = tok * n_cores       # global token count for BN stats
    NH = nt // 4                    # half-groups (4 tiles each)

    nc = bacc.Bacc("TRN2", target_bir_lowering=False, debug=False,
                   dynamic_dma_scratch_size=65536)

    x = nc.declare_dram_parameter("x", [tok, D], F32, isOutput=False)
    cl = nc.declare_dram_parameter("clusters", [D, KG], F32, isOutput=False)
    c2 = nc.declare_dram_parameter("clusters2", [D, K], F32, isOutput=False)
    gam = nc.declare_dram_parameter("bn_gamma", [1, KG], F32, isOutput=False)
    bet = nc.declare_dram_parameter("bn_beta", [1, KG], F32, isOutput=False)
    y = nc.declare_dram_parameter("y", [b_loc, D * K], F32, isOutput=True)

    eye_c = nc.inline_tensor(np.eye(128, dtype=np.float16), name="c_eye")

    with tile.TileContext(nc) as tc:
        with (
            tc.tile_pool(name="persist", bufs=1) as persist,
            tc.tile_pool(name="work", bufs=4) as work,
            tc.tile_pool(name="dram", bufs=1, space="DRAM") as dram,
        ):
            # ---- persistent SBUF tensors ----
            xh = persist.tile([128, nt, D], F16, name="xh")
            assn = persist.tile([128, nt, KG], F16, name="assn")
            asq = persist.tile([128, nt, KG], F16, name="asq")
            sm = persist.tile([128, nt, K], F16, name="sm")
            idn = persist.tile([128, 128], F16, name="idn")
            clh = persist.tile([128, 4, KG], F16, name="clh")
            c2n = persist.tile([128, 4, K], F16, name="c2n")
            ones16 = persist.tile([128, 1], F16, name="ones16")
            ones_row = persist.tile([1, 128], F16, name="ones_row")
            gamma = persist.tile([1, KG], F32, name="gamma")
            beta = persist.tile([1, KG], F32, name="beta")
            ss16 = persist.tile([1, 2 * KG], F16, name="ss16")
            bcB = persist.tile([128, 2 * KG], F16, name="bcB")
            stats_sb = persist.tile([1, 2 * KG], F32, name="stats_sb")
            stats_g = persist.tile([1, 2 * KG], F32, name="stats_g")
            actwarm = persist.tile([1, 1], F32, name="actwarm")
            eps_sb = persist.tile([1, 1], F32, name="eps_sb")

            stats_in = dram.tile([1, 2 * KG], F32, name="stats_in")
            stats_out = dram.tile([1, 2 * KG], F32, name="stats_out")

            # ---- phase 0: constants + x load/cast ----
            nc.sync.dma_start(gamma[:], gam[:, :])
            nc.sync.dma_start(beta[:], bet[:, :])
            nc.sync.dma_start(idn[:], eye_c.ap()[:, :])
            nc.vector.memset(ones16[:], 1.0)
            nc.vector.memset(ones_row[:], 1.0)
            nc.vector.memset(eps_sb[:], BN_EPS)
            # Pre-load the one activation table covering every function this
            # kernel uses (ln/exp/copy/square), so the table-load inserter
            # doesn't alternate between ln-only and exp-only sets. Best
            # effort: fall back to automatic insertion if the set is absent.
            try:
                from concourse.hw_specs import get_activation_tables
                tabs = get_activation_tables(nc.m.arch)
                set_id = list(tabs).index("natural_log_exp_and_others")
                nc.scalar.add_instruction(mybir.InstLoadActFuncSet(
                    name=nc.get_next_instruction_name(),
                    engine=mybir.EngineType.Activation,
                    act_func_set_id=set_id, ins=[], outs=[]))
            except (ImportError, ValueError, KeyError):
                pass
            # Touch the activation engine early so any residual table load
            # happens off the critical path.
            nc.scalar.activation(actwarm[:], gamma[:, :1], ACTF.Ln)

            # x cast-DMA (SWDGE casts fp32->fp16 in the DMA engines; HBM
            # read is the real cost). Small first chunks start the PE
            # transpose pipeline sooner.
            xr = x.ap().rearrange("(t p) d -> p t d", p=128)
            t0 = 0
            for sz in LOADS + (8,) * ((nt - sum(LOADS)) // 8):
                nc.gpsimd.dma_start(
                    xh[:, t0:t0 + sz, :], xr[:, t0:t0 + sz, :])
                t0 += sz
            assert t0 == nt
            # clusters via HWDGE (fp32) + DVE cast: the Pool/SWDGE queue is
            # saturated by the x loads, and clh is needed early.
            clf = work.tile([128, 4, KG], F32, name="clf", tag="clf", bufs=1)
            nc.sync.dma_start(
                clf[:], cl.ap().rearrange("(c p) k -> p c k", p=128))
            nc.vector.tensor_copy(clh[:], clf[:])
            # clusters2 in d=4p+c layout (matches vlad output partitioning);
            # not needed until the post stage, so SWDGE order is fine.
            nc.gpsimd.dma_start(
                c2n[:], c2.ap().rearrange("(p c) k -> p c k", c=4))

            # ---- phase A: transposes + assignment matmul + BN stats ----
            with tc.tile_pool(name="psA", bufs=2, space="PSUM") as psA:
                # separate banks so the token-sum group can run while the
                # sum-of-squares group is still accumulating (start=True
                # clears a whole bank's has_written bits)
                pstat_q = psA.tile([1, KG], F32, name="pstat_q",
                                   tag="st_q", bufs=1)
                pstat_s = psA.tile([1, KG], F32, name="pstat_s",
                                   tag="st_s", bufs=1)

                NQ = nt // 2            # quarter-groups (2 tiles each)
                xtbufs = {}
                p1bufs = {}

                def produce(q):
                    # xTsb for quarter q: [128, 8, 128] fp16 with block
                    # e = 4j + c holding x[tile 2q+j, 128c:128c+128]^T
                    xTsb = work.tile([128, 8, 128], F16, name=f"xT{q}",
                                     tag="xt", bufs=LAG + 2)
                    if q in XBAR_QUARTERS:
                        nc.sync.dma_start(xTsb[:, :, :],
                                          xh[:, 2 * q:2 * (q + 1), :],
                                          transpose=True)
                    else:
                        pxt = psA.tile([128, 8, 128], F16, name="pxt",
                                       tag="pxt", bufs=PXT_BUFS)
                        for j in range(2):
                            t = 2 * q + j
                            for c in range(4):
                                nc.tensor.transpose(
                                    pxt[:, 4 * j + c, :],
                                    xh[:, t, 128 * c:128 * (c + 1)], idn[:])
                        # batched PSUM->SBUF evacuation; alternate DVE/Act
                        if q % 2 == 0:
                            nc.vector.tensor_copy(xTsb[:], pxt[:])
                        else:
                            nc.scalar.activation(xTsb[:], pxt[:], ACTF.Copy)
                    xtbufs[q] = xTsb

                def consume(q):
                    xTsb = xtbufs.pop(q)
                    if q % 2 == 0:
                        p1bufs[q // 2] = psA.tile([128, 4, KG], F32,
                                                  name="p1", tag="p1", bufs=2)
                    p1 = p1bufs[q // 2]
                    for j in range(2):
                        for c in range(4):
                            nc.tensor.matmul(
                                p1[:, 2 * (q % 2) + j, :],
                                xTsb[:, 4 * j + c, :],
                                clh[:, c, :], start=(c == 0), stop=(c == 3),
                                skip_group_check=True)
                    if q % 2 == 1:
                        h = q // 2
                        sl = slice(4 * h, 4 * (h + 1))
                        nc.scalar.activation(assn[:, sl, :], p1[:], ACTF.Copy)
                        if h == NH - 1:
                            # Act square: DVE is backlogged at phase-A end
                            with nc.allow_low_precision("fp16 stats sq"):
                                nc.scalar.activation(asq[:, sl, :],
                                                     assn[:, sl, :],
                                                     ACTF.Square)
                        else:
                            nc.vector.tensor_tensor(asq[:, sl, :],
                                                    assn[:, sl, :],
                                                    assn[:, sl, :],
                                                    op=OP.mult)

                def stats(h):
                    for j in range(4):
                        t = 4 * h + j
                        nc.tensor.matmul(pstat_q[:], ones16[:],
                                         asq[:, t, :],
                                         start=(t == 0), stop=(t == nt - 1),
                                         skip_group_check=True)
                        if t >= 3 * nt // 4:
                            nc.tensor.matmul(pstat_s[:], ones16[:],
                                             assn[:, t, :],
                                             start=(t == 3 * nt // 4),
                                             stop=False,
                                             skip_group_check=True)

                sacc = persist.tile([128, KG], F16, name="sacc")

                def ssum(c):
                    # DVE free-axis partial sum of assn over 16 tiles
                    with nc.allow_low_precision("fp16 stats partials"):
                        if c == 0:
                            nc.vector.tensor_reduce(
                                sacc[:],
                                assn[:, :16, :].rearrange("p t k -> p k t"),
                                axis=AX.X, op=OP.add)
                            return
                        red = work.tile([128, KG], F16, name="red", tag="red",
                                        bufs=2)
                        nc.vector.tensor_reduce(
                            red[:],
                            assn[:, 16 * c:16 * (c + 1), :]
                            .rearrange("p t k -> p k t"),
                            axis=AX.X, op=OP.add)
                        nc.vector.tensor_tensor(sacc[:], sacc[:], red[:],
                                                op=OP.add)

                # Stats matmuls are emitted 3 half-groups behind the assn
                # evacuations they read: the PE queue is in-order, so a stats
                # matmul whose Act/DVE evacuation hasn't retired yet would
                # stall the whole PE pipeline.
                stats_done = 0
                ssum_done = 0
                for q in range(NQ + LAG):
                    if q < NQ:
                        produce(q)
                    if q >= LAG:
                        cq = q - LAG
                        consume(cq)
                        ready_h = (cq + 1) // 2 - 3
                        while stats_done < ready_h:
                            stats(stats_done)
                            stats_done += 1
                        while ssum_done < min(3, ready_h // 4):
                            ssum(ssum_done)
                            ssum_done += 1
                while stats_done < NH:
                    stats(stats_done)
                    stats_done += 1
                while ssum_done < 3:
                    ssum(ssum_done)
                    ssum_done += 1
                # close the token-sum group with the DVE partial (tiles 0-47)
                nc.tensor.matmul(pstat_s[:], ones16[:], sacc[:],
                                 start=False, stop=True,
                                 skip_group_check=True)

                # ---- neck: stats all-reduce + BN parameters ----
                # stats_sb layout: [sum_sq (q), sum (s)]
                nc.vector.tensor_copy(stats_sb[:, :KG], pstat_q[:])
                nc.vector.tensor_copy(stats_sb[:, KG:], pstat_s[:])

            if with_collective:
                nc.sync.dma_start(stats_in[:], stats_sb[:])
                nc.gpsimd.collective_compute(
                    "AllReduce", OP.add,
                    replica_groups=[list(range(n_cores))],
                    ins=[stats_in.opt()], outs=[stats_out.opt()])
                nc.sync.dma_start(stats_g[:], stats_out[:])
            else:
                # single-core stand-in for the collective hop
                nc.sync.dma_start(stats_g[:], stats_sb[:])

            t_s2 = work.tile([1, KG], F32, name="t_s2", tag="sv2", bufs=4)
            t_vr = work.tile([1, KG], F32, name="t_vr", tag="sv2", bufs=4)
            t_ln = work.tile([1, KG], F32, name="t_ln", tag="sv2", bufs=4)
            t_sc = work.tile([1, KG], F32, name="t_sc", tag="sv2", bufs=4)
            t_mc = work.tile([1, KG], F32, name="t_mc", tag="sv2", bufs=4)
            inv_n = 1.0 / float(total_tok)
            # var = inv_n*(q - inv_n*s^2); rsqrt via exp(-0.5 ln(.)) with the
            # inv_n factor folded into the Ln's scale operand
            q_row, s_row = stats_g[:, :KG], stats_g[:, KG:]
            nc.vector.tensor_tensor(t_s2[:], s_row, s_row, op=OP.mult)
            nc.vector.scalar_tensor_tensor(t_vr[:], t_s2[:], -inv_n, q_row,
                                           op0=OP.mult, op1=OP.add)
            nc.scalar.activation(t_ln[:], t_vr[:], ACTF.Ln, bias=eps_sb[:],
                                 scale=inv_n)
            nc.scalar.activation(t_sc[:], t_ln[:], ACTF.Exp, scale=-0.5)
            with nc.allow_low_precision("fp16 bn scale"):
                nc.vector.tensor_tensor(ss16[:, :KG], t_sc[:], gamma[:],
                                        op=OP.mult)
            # shift = beta - (inv_n*s)*scale_f32*gamma; use fp16 scale copy
            with nc.allow_low_precision("fp16 bn shift"):
                nc.vector.scalar_tensor_tensor(t_mc[:], s_row, inv_n,
                                               ss16[:, :KG],
                                               op0=OP.mult, op1=OP.mult)
                nc.vector.tensor_tensor(ss16[:, KG:], beta[:], t_mc[:],
                                        op=OP.subtract)
            nc.gpsimd.partition_broadcast(bcB[:], ss16[:])
            scale_b = bcB[:, :KG].rearrange("p (a k) -> p a k", a=1)
            shift_b = bcB[:, KG:].rearrange("p (a k) -> p a k", a=1)

            # ---- phase BC: softmax + vlad + normalize, per batch ----
            with (
                tc.tile_pool(name="psB", bufs=2, space="PSUM") as psB,
                tc.tile_pool(name="elem", bufs=2) as elem,
                tc.tile_pool(name="vpost", bufs=2) as vpost,
            ):
                state = {}
                tebufs = {}

                def te_chunk(t0, n, pool_add=False):
                    # te = exp(scale*assn + shift) for token tiles [t0,t0+n)
                    te = elem.tile([128, n, KG], F16, name="te",
                                   tag=f"te{t0}_{n}", bufs=1)
                    nc.vector.tensor_tensor(
                        te[:], assn[:, t0:t0 + n, :],
                        scale_b.to_broadcast([128, n, KG]), op=OP.mult)
                    eng = nc.gpsimd if pool_add else nc.vector
                    eng.tensor_tensor(
                        te[:], te[:], shift_b.to_broadcast([128, n, KG]),
                        op=OP.add)
                    nc.scalar.activation(te[:], te[:], ACTF.Exp)
                    tebufs[t0] = te

                def sm_chunk(t0, n):
                    # normalize: sm = te / sum_k te, dropping ghosts
                    te = tebufs.pop(t0)
                    # pairwise-add tree at fp16 2x before the 1x reduce
                    dh = work.tile([128, n, KG // 2], F16, name="dh",
                                   tag=f"dh{n}", bufs=3)
                    dh2 = work.tile([128, n, KG // 4], F16, name="dh2",
                                    tag=f"dh2{n}", bufs=3)
                    with nc.allow_low_precision("fp16 softmax denom"):
                        nc.vector.tensor_tensor(dh[:], te[:, :, :KG // 2],
                                                te[:, :, KG // 2:], op=OP.add)
                        nc.vector.tensor_tensor(dh2[:], dh[:, :, :KG // 4],
                                                dh[:, :, KG // 4:], op=OP.add)
                    denom = work.tile([128, n], F16, name="denom", tag=f"dn{n}",
                                      bufs=3)
                    with nc.allow_low_precision("fp16 softmax denom"):
                        nc.vector.tensor_reduce(denom[:], dh2[:], axis=AX.X,
                                                op=OP.add)
                    recip = work.tile([128, n], F16, name="recip", tag=f"rc{n}",
                                      bufs=3)
                    with nc.allow_low_precision("fp16 softmax recip"):
                        nc.vector.reciprocal(recip[:], denom[:])
                    nc.vector.tensor_tensor(
                        sm[:, t0:t0 + n, :], te[:, :, :K],
                        recip[:].rearrange("p (t a) -> p t a", a=1)
                        .to_broadcast([128, n, K]), op=OP.mult)

                # chunking: small first chunks so the first vlad matmuls can
                # start early; full batches later for low op overhead
                CHUNKS = [(0, 1), (1, 1), (2, 2), (4, 4), (8, 8)] + [
                    (b * TPB, TPB) for b in range(1, b_loc - 1)] + [
                    ((b_loc - 1) * TPB, TPB // 2),
                    ((b_loc - 1) * TPB + TPB // 2, TPB // 2)]

                def mm_stage(b):
                    t0 = b * TPB
                    pv = psB.tile([128, 4, K], F32, name="pv", tag="pv")
                    pas = psB.tile([1, K], F32, name="pas", tag="pas")
                    # a_sum first: its PSUM lands while the vlad c-groups
                    # stream, so av is ready before the last c-group stops
                    for i in range(TPB):
                        nc.tensor.matmul(pas[:], ones16[:], sm[:, t0 + i, :],
                                         start=(i == 0), stop=(i == TPB - 1),
                                         skip_group_check=True)
                    pa16 = work.tile([1, K], F16, name="pa16", tag="pa16",
                                     bufs=2)
                    with nc.allow_low_precision("fp16 a_sum"):
                        nc.scalar.activation(pa16[:], pas[:], ACTF.Copy)
                    av = vpost.tile([128, 4, K], F16, name="av", tag="av")
                    if b == b_loc - 1:
                        # last batch: broadcast via PE + DVE to skip the Pool
                        # round-trips on the tail-critical path
                        pamP = psB.tile([128, K], F32, name="pamP", tag="pamP")
                        nc.tensor.matmul(pamP[:], ones_row[:], pa16[:],
                                         start=True, stop=True,
                                         skip_group_check=True)
                        nc.vector.tensor_tensor(
                            av[:], c2n[:],
                            pamP[:].rearrange("p (a k) -> p a k", a=1)
                            .to_broadcast([128, 4, K]), op=OP.mult)
                    else:
                        pamB = vpost.tile([128, K], F16, name="pamB",
                                          tag="pam")
                        nc.gpsimd.partition_broadcast(pamB[:], pa16[:])
                        nc.gpsimd.tensor_tensor(
                            av[:], c2n[:],
                            pamB[:].rearrange("p (a k) -> p a k", a=1)
                            .to_broadcast([128, 4, K]), op=OP.mult)
                    # vlad: x stationary with d = 4p + c column layout
                    for c in range(4):
                        for i in range(TPB):
                            t = t0 + i
                            nc.tensor.matmul(
                                pv[:, c, :],
                                xh[:, t, c::4],
                                sm[:, t, :],
                                start=(i == 0), stop=(i == TPB - 1),
                                skip_group_check=True)
                    state[b] = (pv, av)

                def post_stage(b):
                    pv, av = state.pop(b)
                    v = vpost.tile([128, 4, K], F16, name="v", tag="v")
                    sq = vpost.tile([128, 4, K], F16, name="sq", tag="sq")
                    pnrm = psB.tile([1, K], F32, name="pnrm", tag="pnrm")
                    # halves over the c dim: v/sq/pnrm for c<2 overlap the
                    # c2/c3 vlad matmuls of this batch
                    for hc in range(2):
                        cs = slice(2 * hc, 2 * hc + 2)
                        with nc.allow_low_precision("fp16 vlad residual"):
                            nc.vector.tensor_tensor(v[:, cs, :], pv[:, cs, :],
                                                    av[:, cs, :],
                                                    op=OP.subtract)
                        with nc.allow_low_precision("fp16 norm squares"):
                            nc.scalar.activation(sq[:, cs, :], v[:, cs, :],
                                                 ACTF.Square)
                        for c in range(2 * hc, 2 * hc + 2):
                            nc.tensor.matmul(pnrm[:], ones16[:], sq[:, c, :],
                                             start=(c == 0), stop=(c == 3),
                                             skip_group_check=True)
                    # y = v * rsqrt(64*nrm2): intra-norm and global L2 norm
                    # folded (flat norm is exactly sqrt(64) post intra-norm)
                    rnl = work.tile([1, K], F32, name="rnl", tag="rnl")
                    nc.scalar.activation(rnl[:], pnrm[:], ACTF.Ln, scale=64.0)
                    rn16 = work.tile([1, K], F16, name="rn16", tag="rn16")
                    with nc.allow_low_precision("fp16 norm scale"):
                        nc.scalar.activation(rn16[:], rnl[:], ACTF.Exp,
                                             scale=-0.5)
                    vf = vpost.tile([128, 4, K], F32, name="vf", tag="vf")
                    yb = y[b, :].rearrange("(p c k) -> p c k", p=128, k=K)
                    if b == b_loc - 1:
                        # last batch: PE broadcast + DVE scale + split y
                        # write to shorten the tail-critical chain
                        prnP = psB.tile([128, K], F32, name="prnP", tag="pamP")
                        nc.tensor.matmul(prnP[:], ones_row[:], rn16[:],
                                         start=True, stop=True,
                                         skip_group_check=True)
                        prnPv = prnP[:].rearrange("p (a k) -> p a k", a=1)
                        for hc in range(2):
                            cs = slice(2 * hc, 2 * hc + 2)
                            nc.vector.tensor_tensor(
                                vf[:, cs, :], v[:, cs, :],
                                prnPv.to_broadcast([128, 2, K]), op=OP.mult)
                            nc.sync.dma_start(yb[:, cs, :], vf[:, cs, :])
                    else:
                        prnB = vpost.tile([128, K], F16, name="prnB",
                                          tag="prn")
                        nc.gpsimd.partition_broadcast(prnB[:], rn16[:])
                        prnBv = prnB[:].rearrange("p (a k) -> p a k", a=1)
                        nc.gpsimd.tensor_tensor(
                            vf[:], v[:], prnBv.to_broadcast([128, 4, K]),
                            op=OP.mult)
                        nc.sync.dma_start(yb[:, :, :], vf[:])

                # Skew-by-one software pipeline: each sm chunk is emitted one
                # te-chunk later so the DVE stream never waits on an Act exp;
                # vlad (PE) and post stages weave in as batches complete.
                nch = len(CHUNKS)
                done_b = 0
                for i in range(nch + 1):
                    if i < nch:
                        t0, n = CHUNKS[i]
                        te_chunk(t0, n, pool_add=False)
                    if i >= 1:
                        t0, n = CHUNKS[i - 1]
                        sm_chunk(t0, n)
                        if (t0 + n) % TPB == 0:     # batch done_b fully sm'd
                            mm_stage(done_b)
                            if done_b >= 1:
                                post_stage(done_b - 1)
                            done_b += 1
                post_stage(b_loc - 1)
    nc.compile()
    return nc


_CACHE = {}


def _get(b_loc, n_cores, with_collective):
    key = (b_loc, n_cores, with_collective)
    if key not in _CACHE:
        _CACHE[key] = build(b_loc, n_cores, with_collective)
    return _CACHE[key]


def make_in_maps(x, clusters, clusters2, bn_gamma, bn_beta, n_cores=N_CORES):
    B = x.shape[0]
    b_loc = B // n_cores
    shared = {
        "clusters": np.ascontiguousarray(clusters, np.float32),
        "clusters2": np.ascontiguousarray(
            np.asarray(clusters2).reshape(D, K), np.float32),
        "bn_gamma": np.ascontiguousarray(
            np.asarray(bn_gamma).reshape(1, KG), np.float32),
        "bn_beta": np.ascontiguousarray(
            np.asarray(bn_beta).reshape(1, KG), np.float32),
    }
    in_maps = []
    for i in range(n_cores):
        m = dict(shared)
        m["x"] = np.ascontiguousarray(
            np.asarray(x[i * b_loc:(i + 1) * b_loc]).reshape(
                b_loc * N_SEQ, D), np.float32)
        in_maps.append(m)
    return in_maps


def kernel(x, clusters, clusters2, bn_gamma, bn_beta):
    B, N, Dd = x.shape
    assert (N, Dd) == (N_SEQ, D) and B % N_CORES == 0
    b_loc = B // N_CORES
    nc = _get(b_loc, N_CORES, True)
    in_maps = make_in_maps(x, clusters, clusters2, bn_gamma, bn_beta)
    res = run_bass_kernel_spmd(nc, in_maps, core_ids=list(range(N_CORES)))
    out = np.concatenate([res.results[i]["y"] for i in range(N_CORES)], axis=0)
    return out


# revision 57
# speedup vs baseline: 2.0882x; 1.0055x over previous
"""NetVLAD-style vq_codebook kernel for 8 Trainium2 NeuronCores.

Reference computation (per full input):
  assn = BN(x @ clusters); softmax over 80 clusters, drop 16 ghosts
  vlad[b,d,k] = sum_n assn[b,n,k] x[b,n,d] - a_sum[b,k]*clusters2[d,k]
  intra-normalize over d, flatten, global L2 normalize -> (B, D*K)

Sharding: data-parallel over batch B (B/8 batches per core). BatchNorm
statistics (sum and sum-of-squares per cluster column) are all-reduced
across the 8 cores (2*80 floats). Everything else is local.

Key structure (v2, redesigned around the engine cost model):
 - x cast-loaded fp32->fp16 by SWDGE DMA in token-partition layout.
 - x^T (d-partition) via PE transposes (is_transpose matmuls writing
   fp16 PSUM), software-pipelined at 2-tile granularity and batch-
   evacuated to SBUF by DVE and Act in a ~17:15 split.
 - assignment matmul per token tile: 4 accumulating (128x128)@(128x80)
   fp16 matmuls. BN sum-of-squares via a long PE ones-matmul group;
   BN sums via DVE free-axis reduces + a PE partition reduce, sharing
   one PSUM bank with strictly sequential accumulation groups.
 - softmax: scale/shift as fp16 2x DVE tensor-tensor ops, Exp on Act
   (one activation table for ln/exp/copy/square -> a single load),
   pairwise-halved fp16 denominator, 1/sqrt as exp(-0.5*ln(x)).
 - vlad with x stationary in a d=4p+c column layout so the final DMA
   writes 1KB-contiguous runs; a_sum accumulated directly as [1,64]
   before the vlad groups so a_sum*clusters2 overlaps them.
 - global L2 norm folded analytically: after intra-normalization the
   flat norm is exactly sqrt(64), so y = v * rsqrt(64*nrm2[k]).
 - one serial neck (stats hop + BN math) between the assignment pass
   and the softmax/vlad pass; batch-0 softmax runs in small chunks so
   the first vlad matmuls start early.
"""

import sys

for _p in ("/opt/trn_rl_repo", "/root/.axon_site/_ro/trn_rl_repo"):
    if _p not in sys.path:
        sys.path.insert(0, _p)

import numpy as np

import concourse.bacc as bacc
import concourse.mybir as mybir
import concourse.tile as tile
from concourse.bass_utils import run_bass_kernel_spmd

F32 = mybir.dt.float32
F16 = mybir.dt.float16
AX = mybir.AxisListType
OP = mybir.AluOpType
ACTF = mybir.ActivationFunctionType

N_CORES = 8
D = 512
KG = 80          # clusters + ghosts
K = 64           # real clusters
N_SEQ = 2048
TPB = N_SEQ // 128   # token tiles per batch = 16
BN_EPS = 1e-5

# Tunables
import os as _os
XBAR_QUARTERS = tuple(
    int(v) for v in _os.environ.get("K_XBAR", "").split(",") if v)
LAG = int(_os.environ.get("K_LAG", "5"))
LOADS = tuple(int(v) for v in _os.environ.get("K_LOADS", "4,4").split(","))
PXT_BUFS = int(_os.environ.get("K_PXT", "4"))


def build(b_loc=4, n_cores=N_CORES, with_collective=True):
    """Build the per-core program. b_loc = batches per core."""
    nt = b_loc * TPB                # token tiles per core
    tok = nt * 128                  # tokens per core
    total_tok = tok * n_cores       # global token count for BN stats
    NH = nt // 4                    # half-groups (4 tiles each)

    nc = bacc.Bacc("TRN2", target_bir_lowering=False, debug=False,
                   dynamic_dma_scratch_size=65536)

    x = nc.declare_dram_parameter("x", [tok, D], F32, isOutput=False)
    cl = nc.declare_dram_parameter("clusters", [D, KG], F32, isOutput=False)
    c2 = nc.declare_dram_parameter("clusters2", [D, K], F32, isOutput=False)
    gam = nc.declare_dram_parameter("bn_gamma", [1, KG], F32, isOutput=False)
    bet = nc.declare_dram_parameter("bn_beta", [1, KG], F32, isOutput=False)
    y = nc.declare_dram_parameter("y", [b_loc, D * K], F32, isOutput=True)

    eye_c = nc.inline_tensor(np.eye(128, dtype=np.float16), name="c_eye")

    with tile.TileContext(nc) as tc:
        with (
            tc.tile_pool(name="persist", bufs=1) as persist,
            tc.tile_pool(name="work", bufs=4) as work,
            tc.tile_pool(name="dram", bufs=1, space="DRAM") as dram,
        ):
            # ---- persistent SBUF tensors ----
            xh = persist.tile([128, nt, D], F16, name="xh")
            assn = persist.tile([128, nt, KG], F16, name="assn")
            asq = persist.tile([128, nt, KG], F16, name="asq")
            sm = persist.tile([128, nt, K], F16, name="sm")
            idn = persist.tile([128, 128], F16, name="idn")
            clh = persist.tile([128, 4, KG], F16, name="clh")
            c2n = persist.tile([128, 4, K], F16, name="c2n")
            ones16 = persist.tile([128, 1], F16, name="ones16")
            ones_row = persist.tile([1, 128], F16, name="ones_row")
            gamma = persist.tile([1, KG], F32, name="gamma")
            beta = persist.tile([1, KG], F32, name="beta")
            ss16 = persist.tile([1, 2 * KG], F16, name="ss16")
            bcB = persist.tile([128, 2 * KG], F16, name="bcB")
            stats_sb = persist.tile([1, 2 * KG], F32, name="stats_sb")
            stats_g = persist.tile([1, 2 * KG], F32, name="stats_g")
            actwarm = persist.tile([1, 1], F32, name="actwarm")
            eps_sb = persist.tile([1, 1], F32, name="eps_sb")

            stats_in = dram.tile([1, 2 * KG], F32, name="stats_in")
            stats_out = dram.tile([1, 2 * KG], F32, name="stats_out")

            # ---- phase 0: constants + x load/cast ----
            nc.sync.dma_start(gamma[:], gam[:, :])
            nc.sync.dma_start(beta[:], bet[:, :])
            nc.sync.dma_start(idn[:], eye_c.ap()[:, :])
            nc.vector.memset(ones16[:], 1.0)
            nc.vector.memset(ones_row[:], 1.0)
            nc.vector.memset(eps_sb[:], BN_EPS)
            # Pre-load the one activation table covering every function this
            # kernel uses (ln/exp/copy/square), so the table-load inserter
            # doesn't alternate between ln-only and exp-only sets. Best
            # effort: fall back to automatic insertion if the set is absent.
            try:
                from concourse.hw_specs import get_activation_tables
                tabs = get_activation_tables(nc.m.arch)
                set_id = list(tabs).index("natural_log_exp_and_others")
                nc.scalar.add_instruction(mybir.InstLoadActFuncSet(
                    name=nc.get_next_instruction_name(),
                    engine=mybir.EngineType.Activation,
                    act_func_set_id=set_id, ins=[], outs=[]))
            except (ImportError, ValueError, KeyError):
                pass
            # Touch the activation engine early so any residual table load
            # happens off the critical path.
            nc.scalar.activation(actwarm[:], gamma[:, :1], ACTF.Ln)

            # x cast-DMA (SWDGE casts fp32->fp16 in the DMA engines; HBM
            # read is the real cost). Small first chunks start the PE
            # transpose pipeline sooner.
            xr = x.ap().rearrange("(t p) d -> p t d", p=128)
            t0 = 0
            for sz in LOADS + (8,) * ((nt - sum(LOADS)) // 8):
                nc.gpsimd.dma_start(
                    xh[:, t0:t0 + sz, :], xr[:, t0:t0 + sz, :])
                t0 += sz
            assert t0 == nt
            # clusters via HWDGE (fp32) + DVE cast: the Pool/SWDGE queue is
            # saturated by the x loads, and clh is needed early.
            clf = work.tile([128, 4, KG], F32, name="clf", tag="clf", bufs=1)
            nc.sync.dma_start(
                clf[:], cl.ap().rearrange("(c p) k -> p c k", p=128))
            nc.vector.tensor_copy(clh[:], clf[:])
            # clusters2 in d=4p+c layout (matches vlad output partitioning);
            # not needed until the post stage, so SWDGE order is fine.
            nc.gpsimd.dma_start(
                c2n[:], c2.ap().rearrange("(p c) k -> p c k", c=4))

            # ---- phase A: transposes + assignment matmul + BN stats ----
            with tc.tile_pool(name="psA", bufs=2, space="PSUM") as psA:
                # separate banks so the token-sum group can run while the
                # sum-of-squares group is still accumulating (start=True
                # clears a whole bank's has_written bits)
                pstat_q = psA.tile([1, KG], F32, name="pstat_q",
                                   tag="st_q", bufs=1)
                pstat_s = psA.tile([1, KG], F32, name="pstat_s",
                                   tag="st_s", bufs=1)

                NQ = nt // 2            # quarter-groups (2 tiles each)
                xtbufs = {}
                p1bufs = {}

                def produce(q):
                    # xTsb for quarter q: [128, 8, 128] fp16 with block
                    # e = 4j + c holding x[tile 2q+j, 128c:128c+128]^T
                    xTsb = work.tile([128, 8, 128], F16, name=f"xT{q}",
                                     tag="xt", bufs=LAG + 2)
                    if q in XBAR_QUARTERS:
                        nc.sync.dma_start(xTsb[:, :, :],
                                          xh[:, 2 * q:2 * (q + 1), :],
                                          transpose=True)
                    else:
                        pxt = psA.tile([128, 8, 128], F16, name="pxt",
                                       tag="pxt", bufs=PXT_BUFS)
                        for j in range(2):
                            t = 2 * q + j
                            for c in range(4):
                                nc.tensor.transpose(
                                    pxt[:, 4 * j + c, :],
                                    xh[:, t, 128 * c:128 * (c + 1)], idn[:])
                        # batched PSUM->SBUF evacuation; alternate DVE/Act
                        if q % 2 == 0:
                            nc.vector.tensor_copy(xTsb[:], pxt[:])
                        else:
                            nc.scalar.activation(xTsb[:], pxt[:], ACTF.Copy)
                    xtbufs[q] = xTsb

                def consume(q):
                    xTsb = xtbufs.pop(q)
                    if q % 2 == 0:
                        p1bufs[q // 2] = psA.tile([128, 4, KG], F32,
                                                  name="p1", tag="p1", bufs=2)
                    p1 = p1bufs[q // 2]
                    for j in range(2):
                        for c in range(4):
                            nc.tensor.matmul(
                                p1[:, 2 * (q % 2) + j, :],
                                xTsb[:, 4 * j + c, :],
                                clh[:, c, :], start=(c == 0), stop=(c == 3),
                                skip_group_check=True)
                    if q % 2 == 1:
                        h = q // 2
                        sl = slice(4 * h, 4 * (h + 1))
                        nc.scalar.activation(assn[:, sl, :], p1[:], ACTF.Copy)
                        if h == NH - 1:
                            # Act square: DVE is backlogged at phase-A end
                            with nc.allow_low_precision("fp16 stats sq"):
                                nc.scalar.activation(asq[:, sl, :],
                                                     assn[:, sl, :],
                                                     ACTF.Square)
                        else:
                            nc.vector.tensor_tensor(asq[:, sl, :],
                                                    assn[:, sl, :],
                                                    assn[:, sl, :],
                                                    op=OP.mult)

                def stats(h):
                    for j in range(4):
                        t = 4 * h + j
                        nc.tensor.matmul(pstat_q[:], ones16[:],
                                         asq[:, t, :],
                                         start=(t == 0), stop=(t == nt - 1),
                                         skip_group_check=True)
                        if t >= 3 * nt // 4:
                            nc.tensor.matmul(pstat_s[:], ones16[:],
                                             assn[:, t, :],
                                             start=(t == 3 * nt // 4),
                                             stop=False,
                                             skip_group_check=True)

                sacc = persist.tile([128, KG], F16, name="sacc")

                def ssum(c):
                    # DVE free-axis partial sum of assn over 16 tiles
                    with nc.allow_low_precision("fp16 stats partials"):
                        if c == 0:
                            nc.vector.tensor_reduce(
                                sacc[:],
                                assn[:, :16, :].rearrange("p t k -> p k t"),
                                axis=AX.X, op=OP.add)
                            return
                        red = work.tile([128, KG], F16, name="red", tag="red",
                                        bufs=2)
                        nc.vector.tensor_reduce(
                            red[:],
                            assn[:, 16 * c:16 * (c + 1), :]
                            .rearrange("p t k -> p k t"),
                            axis=AX.X, op=OP.add)
                        nc.vector.tensor_tensor(sacc[:], sacc[:], red[:],
                                                op=OP.add)

                # Stats matmuls are emitted 3 half-groups behind the assn
                # evacuations they read: the PE queue is in-order, so a stats
                # matmul whose Act/DVE evacuation hasn't retired yet would
                # stall the whole PE pipeline.
                stats_done = 0
                ssum_done = 0
                for q in range(NQ + LAG):
                    if q < NQ:
                        produce(q)
                    if q >= LAG:
                        cq = q - LAG
                        consume(cq)
                        ready_h = (cq + 1) // 2 - 3
                        while stats_done < ready_h:
                            stats(stats_done)
                            stats_done += 1
                        while ssum_done < min(3, ready_h // 4):
                            ssum(ssum_done)
                            ssum_done += 1
                while stats_done < NH:
                    stats(stats_done)
                    stats_done += 1
                while ssum_done < 3:
                    ssum(ssum_done)
                    ssum_done += 1
                # close the token-sum group with the DVE partial (tiles 0-47)
                nc.tensor.matmul(pstat_s[:], ones16[:], sacc[:],
                                 start=False, stop=True,
                                 skip_group_check=True)

                # ---- neck: stats all-reduce + BN parameters ----
                # stats_sb layout: [sum_sq (q), sum (s)]
                nc.vector.tensor_copy(stats_sb[:, :KG], pstat_q[:])
                nc.vector.tensor_copy(stats_sb[:, KG:], pstat_s[:])

            if with_collective:
                nc.sync.dma_start(stats_in[:], stats_sb[:])
                nc.gpsimd.collective_compute(
                    "AllReduce", OP.add,
                    replica_groups=[list(range(n_cores))],
                    ins=[stats_in.opt()], outs=[stats_out.opt()])
                nc.sync.dma_start(stats_g[:], stats_out[:])
            else:
                # single-core stand-in for the collective hop
                nc.sync.dma_start(stats_g[:], stats_sb[:])

            t_s2 = work.tile([1, KG], F32, name="t_s2", tag="sv2", bufs=4)
            t_vr = work.tile([1, KG], F32, name="t_vr", tag="sv2", bufs=4)
            t_ln = work.tile([1, KG], F32, name="t_ln", tag="sv2", bufs=4)
            t_sc = work.tile([1, KG], F32, name="t_sc", tag="sv2", bufs=4)
            t_mc = work.tile([1, KG], F32, name="t_mc", tag="sv2", bufs=4)
            inv_n = 1.0 / float(total_tok)
            # var = inv_n*(q - inv_n*s^2); rsqrt via exp(-0.5 ln(.)) with the
            # inv_n factor folded into the Ln's scale operand
            q_row, s_row = stats_g[:, :KG], stats_g[:, KG:]
            nc.vector.tensor_tensor(t_s2[:], s_row, s_row, op=OP.mult)
            nc.vector.scalar_tensor_tensor(t_vr[:], t_s2[:], -inv_n, q_row,
                                           op0=OP.mult, op1=OP.add)
            nc.scalar.activation(t_ln[:], t_vr[:], ACTF.Ln, bias=eps_sb[:],
                                 scale=inv_n)
            nc.scalar.activation(t_sc[:], t_ln[:], ACTF.Exp, scale=-0.5)
            with nc.allow_low_precision("fp16 bn scale"):
                nc.vector.tensor_tensor(ss16[:, :KG], t_sc[:], gamma[:],
                                        op=OP.mult)
            # shift = beta - (inv_n*s)*scale_f32*gamma; use fp16 scale copy
            with nc.allow_low_precision("fp16 bn shift"):
                nc.vector.scalar_tensor_tensor(t_mc[:], s_row, inv_n,
                                               ss16[:, :KG],
                                               op0=OP.mult, op1=OP.mult)
                nc.vector.tensor_tensor(ss16[:, KG:], beta[:], t_mc[:],
                                        op=OP.subtract)
            nc.gpsimd.partition_broadcast(bcB[:], ss16[:])
            scale_b = bcB[:, :KG].rearrange("p (a k) -> p a k", a=1)
            shift_b = bcB[:, KG:].rearrange("p (a k) -> p a k", a=1)

            # ---- phase BC: softmax + vlad + normalize, per batch ----
            with (
                tc.tile_pool(name="psB", bufs=2, space="PSUM") as psB,
                tc.tile_pool(name="elem", bufs=2) as elem,
                tc.tile_pool(name="vpost", bufs=2) as vpost,
            ):
                state = {}
                tebufs = {}

                def te_chunk(t0, n, pool_add=False):
                    # te = exp(scale*assn + shift) for token tiles [t0,t0+n)
                    te = elem.tile([128, n, KG], F16, name="te",
                                   tag=f"te{t0}_{n}", bufs=1)
                    nc.vector.tensor_tensor(
                        te[:], assn[:, t0:t0 + n, :],
                        scale_b.to_broadcast([128, n, KG]), op=OP.mult)
                    eng = nc.gpsimd if pool_add else nc.vector
                    eng.tensor_tensor(
                        te[:], te[:], shift_b.to_broadcast([128, n, KG]),
                        op=OP.add)
                    nc.scalar.activation(te[:], te[:], ACTF.Exp)
                    tebufs[t0] = te

                def sm_chunk(t0, n):
                    # normalize: sm = te / sum_k te, dropping ghosts
                    te = tebufs.pop(t0)
                    # pairwise-add tree at fp16 2x before the 1x reduce
                    dh = work.tile([128, n, KG // 2], F16, name="dh",
                                   tag=f"dh{n}", bufs=3)
                    dh2 = work.tile([128, n, KG // 4], F16, name="dh2",
                                    tag=f"dh2{n}", bufs=3)
                    with nc.allow_low_precision("fp16 softmax denom"):
                        nc.vector.tensor_tensor(dh[:], te[:, :, :KG // 2],
                                                te[:, :, KG // 2:], op=OP.add)
                        nc.vector.tensor_tensor(dh2[:], dh[:, :, :KG // 4],
                                                dh[:, :, KG // 4:], op=OP.add)
                    denom = work.tile([128, n], F16, name="denom", tag=f"dn{n}",
                                      bufs=3)
                    with nc.allow_low_precision("fp16 softmax denom"):
                        nc.vector.tensor_reduce(denom[:], dh2[:], axis=AX.X,
                                                op=OP.add)
                    recip = work.tile([128, n], F16, name="recip", tag=f"rc{n}",
                                      bufs=3)
                    with nc.allow_low_precision("fp16 softmax recip"):
                        nc.vector.reciprocal(recip[:], denom[:])
                    nc.vector.tensor_tensor(
                        sm[:, t0:t0 + n, :], te[:, :, :K],
                        recip[:].rearrange("p (t a) -> p t a", a=1)
                        .to_broadcast([128, n, K]), op=OP.mult)

                # chunking: small first chunks so the first vlad matmuls can
                # start early; full batches later for low op overhead
                CHUNKS = [(0, 1), (1, 1), (2, 2), (4, 4), (8, 8)] + [
                    (b * TPB, TPB) for b in range(1, b_loc - 1)] + [
                    ((b_loc - 1) * TPB, TPB // 2),
                    ((b_loc - 1) * TPB + TPB // 2, TPB // 2)]

                def mm_stage(b):
                    t0 = b * TPB
                    pv = psB.tile([128, 4, K], F32, name="pv", tag="pv")
                    pas = psB.tile([1, K], F32, name="pas", tag="pas")
                    # a_sum first: its PSUM lands while the vlad c-groups
                    # stream, so av is ready before the last c-group stops
                    for i in range(TPB):
                        nc.tensor.matmul(pas[:], ones16[:], sm[:, t0 + i, :],
                                         start=(i == 0), stop=(i == TPB - 1),
                                         skip_group_check=True)
                    pa16 = work.tile([1, K], F16, name="pa16", tag="pa16",
                                     bufs=2)
                    with nc.allow_low_precision("fp16 a_sum"):
                        nc.scalar.activation(pa16[:], pas[:], ACTF.Copy)
                    av = vpost.tile([128, 4, K], F16, name="av", tag="av")
                    if b == b_loc - 1:
                        # last batch: broadcast via PE + DVE to skip the Pool
                        # round-trips on the tail-critical path
                        pamP = psB.tile([128, K], F32, name="pamP", tag="pamP")
                        nc.tensor.matmul(pamP[:], ones_row[:], pa16[:],
                                         start=True, stop=True,
                                         skip_group_check=True)
                        nc.vector.tensor_tensor(
                            av[:], c2n[:],
                            pamP[:].rearrange("p (a k) -> p a k", a=1)
                            .to_broadcast([128, 4, K]), op=OP.mult)
                    else:
                        pamB = vpost.tile([128, K], F16, name="pamB",
                                          tag="pam")
                        nc.gpsimd.partition_broadcast(pamB[:], pa16[:])
                        nc.gpsimd.tensor_tensor(
                            av[:], c2n[:],
                            pamB[:].rearrange("p (a k) -> p a k", a=1)
                            .to_broadcast([128, 4, K]), op=OP.mult)
                    # vlad: x stationary with d = 4p + c column layout
                    for c in range(4):
                        for i in range(TPB):
                            t = t0 + i
                            nc.tensor.matmul(
                                pv[:, c, :],
                                xh[:, t, c::4],
                                sm[:, t, :],
                                start=(i == 0), stop=(i == TPB - 1),
                                skip_group_check=True)
                    state[b] = (pv, av)

                def post_stage(b):
                    pv, av = state.pop(b)
                    v = vpost.tile([128, 4, K], F16, name="v", tag="v")
                    sq = vpost.tile([128, 4, K], F16, name="sq", tag="sq")
                    pnrm = psB.tile([1, K], F32, name="pnrm", tag="pnrm")
                    # halves over the c dim: v/sq/pnrm for c<2 overlap the
                    # c2/c3 vlad matmuls of this batch
                    for hc in range(2):
                        cs = slice(2 * hc, 2 * hc + 2)
                        with nc.allow_low_precision("fp16 vlad residual"):
                            nc.vector.tensor_tensor(v[:, cs, :], pv[:, cs, :],
                                                    av[:, cs, :],
                                                    op=OP.subtract)
                        with nc.allow_low_precision("fp16 norm squares"):
                            nc.scalar.activation(sq[:, cs, :], v[:, cs, :],
                                                 ACTF.Square)
                        for c in range(2 * hc, 2 * hc + 2):
                            nc.tensor.matmul(pnrm[:], ones16[:], sq[:, c, :],
                                             start=(c == 0), stop=(c == 3),
                                             skip_group_check=True)
                    # y = v * rsqrt(64*nrm2): intra-norm and global L2 norm
                    # folded (flat norm is exactly sqrt(64) post intra-norm)
                    rnl = work.tile([1, K], F32, name="rnl", tag="rnl")
                    nc.scalar.activation(rnl[:], pnrm[:], ACTF.Ln, scale=64.0)
                    rn16 = work.tile([1, K], F16, name="rn16", tag="rn16")
                    with nc.allow_low_precision("fp16 norm scale"):
                        nc.scalar.activation(rn16[:], rnl[:], ACTF.Exp,
                                             scale=-0.5)
                    vf = vpost.tile([128, 4, K], F32, name="vf", tag="vf")
                    yb = y[b, :].rearrange("(p c k) -> p c k", p=128, k=K)
                    if b == b_loc - 1:
                        # last batch: PE broadcast + DVE scale + split y
                        # write to shorten the tail-critical chain
                        prnP = psB.tile([128, K], F32, name="prnP", tag="pamP")
                        nc.tensor.matmul(prnP[:], ones_row[:], rn16[:],
                                         start=True, stop=True,
                                         skip_group_check=True)
                        prnPv = prnP[:].rearrange("p (a k) -> p a k", a=1)
                        for hc in range(2):
                            cs = slice(2 * hc, 2 * hc + 2)
                            nc.vector.tensor_tensor(
                                vf[:, cs, :], v[:, cs, :],
                                prnPv.to_broadcast([128, 2, K]), op=OP.mult)
                            nc.sync.dma_start(yb[:, cs, :], vf[:, cs, :])
                    else:
                        prnB = vpost.tile([128, K], F16, name="prnB",
                                          tag="prn")
                        nc.gpsimd.partition_broadcast(prnB[:], rn16[:])
                        prnBv = prnB[:].rearrange("p (a k) -> p a k", a=1)
                        nc.gpsimd.tensor_tensor(
                            vf[:], v[:], prnBv.to_broadcast([128, 4, K]),
                            op=OP.mult)
                        nc.sync.dma_start(yb[:, :, :], vf[:])

                # Skew-by-one software pipeline: each sm chunk is emitted one
                # te-chunk later so the DVE stream never waits on an Act exp;
                # vlad (PE) and post stages weave in as batches complete.
                nch = len(CHUNKS)
                done_b = 0
                for i in range(nch + 1):
                    if i < nch:
                        t0, n = CHUNKS[i]
                        te_chunk(t0, n, pool_add=False)
                    if i >= 1:
                        t0, n = CHUNKS[i - 1]
                        sm_chunk(t0, n)
                        if (t0 + n) % TPB == 0:     # batch done_b fully sm'd
                            if done_b >= 1:
                                post_stage(done_b - 1)
                            mm_stage(done_b)
                            done_b += 1
                post_stage(b_loc - 1)
    nc.compile()
    return nc


_CACHE = {}


def _get(b_loc, n_cores, with_collective):
    key = (b_loc, n_cores, with_collective)
    if key not in _CACHE:
        _CACHE[key] = build(b_loc, n_cores, with_collective)
    return _CACHE[key]


def make_in_maps(x, clusters, clusters2, bn_gamma, bn_beta, n_cores=N_CORES):
    B = x.shape[0]
    b_loc = B // n_cores
    shared = {
        "clusters": np.ascontiguousarray(clusters, np.float32),
        "clusters2": np.ascontiguousarray(
            np.asarray(clusters2).reshape(D, K), np.float32),
        "bn_gamma": np.ascontiguousarray(
            np.asarray(bn_gamma).reshape(1, KG), np.float32),
        "bn_beta": np.ascontiguousarray(
            np.asarray(bn_beta).reshape(1, KG), np.float32),
    }
    in_maps = []
    for i in range(n_cores):
        m = dict(shared)
        m["x"] = np.ascontiguousarray(
            np.asarray(x[i * b_loc:(i + 1) * b_loc]).reshape(
                b_loc * N_SEQ, D), np.float32)
        in_maps.append(m)
    return in_maps


def kernel(x, clusters, clusters2, bn_gamma, bn_beta):
    B, N, Dd = x.shape
    assert (N, Dd) == (N_SEQ, D) and B % N_CORES == 0
    b_loc = B // N_CORES
    nc = _get(b_loc, N_CORES, True)
    in_maps = make_in_maps(x, clusters, clusters2, bn_gamma, bn_beta)
    res = run_bass_kernel_spmd(nc, in_maps, core_ids=list(range(N_CORES)))
    out = np.concatenate([res.results[i]["y"] for i in range(N_CORES)], axis=0)
    return out
